# revision 22
# baseline (speedup 1.0000x reference)
"""GIN-style GNN encoder (3-layer message passing + FFN + norms + segment pool)
on 8 Trainium2 NeuronCores.

Strategy:
- Nodes are range-partitioned across the 8 cores (5000 each, padded to 5120).
  Edges are assigned to the core owning their *destination* node, so the
  scatter-add is core-local (no all-reduce over node features).
- Per core, local nodes are processed in 40 groups of 128; each group has two
  64-node windows. Edges are bucketed by (src-table-row < 32768) so gather
  indices fit int16, grouped by window, and padded to 128-edge chunks. The
  chunk counts per (bucket, window) are maxed over cores so all 8 cores run
  one identical SPMD program (pad chunks gather row 0 with all-zero
  indicators).
- Gather: dma_gather from a replicated bf16 node table in DRAM (the AllGather
  output of the previous layer). Scatter-add: per chunk, matmul
  (messages[e,f] as stationary)T @ indicator[e,64nodes] accumulating
  aggr^T in PSUM; the indicator carries the edge weight.
- FFN/LayerNorm run with nodes on partitions; BatchNorm statistics are
  one AllReduce of [1,256] per layer; pooling is an indicator matmul into
  per-core partials that the host sums.
"""

import os
import numpy as np
import ml_dtypes

import concourse.bass as bass
import concourse.bacc as bacc
import concourse.mybir as mybir
from concourse.bass_utils import run_bass_kernel_spmd
from concourse.tile import TileContext
from concourse.masks import make_identity
from concourse.tile_rust import add_dep_helper

# Problem constants (hardcoded per harness contract)
N = 40000
E = 640000
F_IN = 128
DIM = 128
L = 3
G = 256
EPS = 1e-5

C = 8            # cores
NPC = 5000       # real nodes per core
NGRP = 40        # groups of 128 (includes padding)
NPAD = NGRP * 128  # 5120
TBL = C * NPAD   # 40960 table rows
BSPLIT = 32768   # int16 bucket split in table-row space
W = 64           # indicator window (nodes)
WPG = 2          # windows per group
NWIN = NGRP * WPG  # 80 windows/core
SC = 8   # chunks per gather super-call (1024 idxs; >=2048 overflows the SWDGE ring and hangs)

BF16 = mybir.dt.bfloat16
F32 = mybir.dt.float32
I16 = mybir.dt.int16

AF = mybir.ActivationFunctionType
ALU = mybir.AluOpType

NP_BF16 = ml_dtypes.bfloat16

_last_run = None  # BassKernelResults of the most recent kernel() call


def _to_np(a, dtype=None):
    a = np.asarray(a)
    if dtype is not None:
        a = a.astype(dtype)
    return a


def _prep(x, src, dst, ew, batch):
    """Host-side sharding. Returns (plan, per-core arrays)."""
    core = dst // NPC
    ldst = dst - core * NPC
    win = ldst // W
    wcol = ldst - win * W
    trow = (src // NPC) * NPAD + (src % NPC)
    bucket = (trow >= BSPLIT).astype(np.int64)

    gid = (core * 2 + bucket) * NWIN + win
    cnt = np.bincount(gid, minlength=C * 2 * NWIN).reshape(C, 2, NWIN)
    cpw = np.maximum(np.ceil(cnt / 128.0).astype(np.int64).max(axis=0), 1)  # [2, NWIN]
    KA = int(cpw[0].sum())
    KB = int(cpw[1].sum())
    K = KA + KB
    chunk_base = np.zeros((2, NWIN), np.int64)
    chunk_base[0, :] = np.concatenate([[0], np.cumsum(cpw[0])[:-1]])
    chunk_base[1, :] = KA + np.concatenate([[0], np.cumsum(cpw[1])[:-1]])

    # per-edge position within its (core,bucket,win) run
    order = np.argsort(gid, kind="stable")
    sg = gid[order]
    runstart = np.zeros(len(sg), np.int64)
    newrun = np.ones(len(sg), bool)
    newrun[1:] = sg[1:] != sg[:-1]
    runidx = np.flatnonzero(newrun)
    runstart[runidx] = runidx
    runstart = np.maximum.accumulate(runstart)
    pos_sorted = np.arange(len(sg)) - runstart
    pos = np.empty(len(sg), np.int64)
    pos[order] = pos_sorted

    chunk = chunk_base[bucket, win] + pos // 128
    prow = pos % 128

    gidx_all = []
    ind_all = []
    for c in range(C):
        m = core == c
        lin = np.zeros(K * 128, np.int32)
        lin[chunk[m] * 128 + prow[m]] = trow[m] - bucket[m] * BSPLIT
        assert lin.max() < 32768 and lin.min() >= 0
        # wrap for dma_gather: index i -> partition i%16, col i//16; replicate x8
        arr16 = lin.astype(np.int16).reshape(K * 8, 16).T  # [16, K*8]
        gidx_all.append(np.tile(arr16, (8, 1)))            # [128, K*8]
        ind = np.zeros((128, K, W), NP_BF16)
        ind[prow[m], chunk[m], wcol[m]] = ew[m].astype(NP_BF16)
        ind_all.append(ind)

    # pooling indicators + node mask
    pool_all = []
    mask_all = []
    for c in range(C):
        pi = np.zeros((128, NGRP, 2, 128), NP_BF16)
        b_loc = batch[c * NPC:(c + 1) * NPC]
        nl = np.arange(NPC)
        t = nl // 128
        p = nl % 128
        pi[p, t, b_loc // 128, b_loc % 128] = 1.0
        pool_all.append(pi)
        mk = np.zeros((128, NGRP), NP_BF16)
        mk[p, t] = 1.0
        mask_all.append(mk)

    # x transposed slices, bf16
    x0T_all = []
    for c in range(C):
        s = np.zeros((128, NPAD), NP_BF16)
        xs = x[c * NPC:(c + 1) * NPC]  # [5000, F_IN]
        # node local n = t*128 + p -> column t*128+p? layout [p, t, f] flattened as
        # [128, NGRP*128] with column t*128+f. We store as [f? no]:
        # lhsT for proj needs [f, node] per tile: columns = node within tile.
        # Use layout [128 f, NPAD nodes]: s[f, n] = x[n, f]
        s[:, :NPC] = xs.T.astype(NP_BF16)
        x0T_all.append(s)

    plan = dict(cpw=cpw, KA=KA, KB=KB, K=K, chunk_base=chunk_base)
    percore = dict(gidx=gidx_all, ind=ind_all, pool=pool_all, mask=mask_all,
                   x0T=x0T_all)
    return plan, percore


def _build(plan, flags, dbg=None):
    """Build the SPMD Bass program (identical across cores)."""
    cpw = plan["cpw"]
    KA, KB, K = plan["KA"], plan["KB"], plan["K"]
    if dbg is None:
        dbg = {}
    NL = dbg.get("nl", L)          # layers to actually run
    DO_FFN = dbg.get("ffn", True)  # run FFN block per group
    DO_TAIL = dbg.get("tail", True)  # stats AR + BN apply + pool + AG
    NO_MM = dbg.get("nomm", False)   # skip seg matmuls/evictions
    NO_IND = dbg.get("noind", False)  # skip indicator DMAs
    NO_CONST = dbg.get("noconst", False)  # skip ident/poolind/mask/eps/ones loads
    NO_PROJ = dbg.get("noproj", False)  # skip proj matmuls (raw x -> table)
    NO_LONGPS = dbg.get("nolongps", False)  # skip stats/pool psum allocation
    DVE_MM = dbg.get("dvemm", False)  # replace seg MMs with DVE consumes
    DUMP = dbg.get("dump", False)    # dump stage intermediates (layer 0)

    nc = bacc.Bacc("TRN2", target_bir_lowering=False)

    # ---- DRAM parameters ----
    p_x0T = nc.declare_dram_parameter("x0T", [128, NPAD], BF16, isOutput=False)
    p_gidx = nc.declare_dram_parameter("gidx", [128, K * 8], I16, isOutput=False)
    p_ind = nc.declare_dram_parameter("ind", [128, K, W], BF16, isOutput=False)
    p_pool = nc.declare_dram_parameter("poolind", [128, NGRP, 2, 128], BF16,
                                       isOutput=False)
    p_mask = nc.declare_dram_parameter("mask", [128, NGRP], BF16, isOutput=False)
    p_wlin = nc.declare_dram_parameter("wlin", [128, 128], BF16, isOutput=False)
    p_blin = nc.declare_dram_parameter("blin", [1, 128], BF16, isOutput=False)
    p_W1 = nc.declare_dram_parameter("W1", [L, 128, 256], BF16, isOutput=False)
    p_b1 = nc.declare_dram_parameter("b1", [L, 1, 256], BF16, isOutput=False)
    # W2 stored as [L, 128, 2, 128]: [:, p, k, :] = W2[l, k*128+p, :]
    p_W2 = nc.declare_dram_parameter("W2", [L, 128, 2, 128], BF16, isOutput=False)
    p_b2 = nc.declare_dram_parameter("b2", [L, 1, 128], BF16, isOutput=False)
    p_lng = nc.declare_dram_parameter("lng", [L, 128, 256], F32, isOutput=False)
    p_lnb = nc.declare_dram_parameter("lnb", [L, 128, 256], F32, isOutput=False)
    p_bng = nc.declare_dram_parameter("bng", [L, 1, 128], F32, isOutput=False)
    p_bnb = nc.declare_dram_parameter("bnb", [L, 1, 128], F32, isOutput=False)
    o_xs = nc.declare_dram_parameter("xs_out", [L, 128, NGRP, 128], F32,
                                     isOutput=True)
    o_pool = nc.declare_dram_parameter("pool_out", [L, 2, 128, 128], F32,
                                       isOutput=True)
    if dbg and dbg.get("dump", False):
        o_dtbl = nc.declare_dram_parameter("dbg_tbl", [TBL, 128], BF16,
                                           isOutput=True)
        o_daggr = nc.declare_dram_parameter("dbg_aggrA", [128, NWIN * W], F32,
                                            isOutput=True)
        o_dysb = nc.declare_dram_parameter("dbg_ysb", [128, NGRP, 128], F32,
                                           isOutput=True)

    # internal DRAM
    ag_in = nc.dram_tensor("ag_in", [NPAD, 128], BF16)
    tables = [nc.dram_tensor(f"table{l}", [TBL, 128], BF16, addr_space="Shared")
              for l in range(L)]
    ar_ins = [nc.dram_tensor(f"ar_in{l}", [1, 256], F32) for l in range(L)]
    ar_outs = [nc.dram_tensor(f"ar_out{l}", [1, 256], F32, addr_space="Shared")
               for l in range(L)]

    RG = [list(range(C))]

    # chunk metadata: list over k of (bucket, window)
    chunk_win = np.zeros(K, np.int64)
    for b in range(2):
        for w in range(NWIN):
            base = plan["chunk_base"][b, w]
            chunk_win[base:base + cpw[b, w]] = w
    # supercalls within each bucket
    def supercalls(lo, hi):
        out = []
        k = lo
        while k < hi:
            out.append((k, min(SC, hi - k)))
            k += SC
        return out

    calls_A = supercalls(0, KA)
    calls_B = supercalls(KA, K)

    with TileContext(nc) as tc:
        with (
            tc.tile_pool(name="const", bufs=1) as cpool,
            tc.tile_pool(name="params", bufs=2) as ppool,
            tc.tile_pool(name="indp", bufs=2) as indp,
            tc.tile_pool(name="goutp", bufs=2) as goutp,
            tc.tile_pool(name="segps", bufs=2, space="PSUM") as segps,
            tc.tile_pool(name="ffnps", bufs=2, space="PSUM") as ffnps,
            tc.tile_pool(name="longps", bufs=1, space="PSUM") as longps,
            tc.tile_pool(name="work", bufs=3) as work,
            tc.tile_pool(name="bigbuf", bufs=1) as bigp,
        ):
            # ---- constants ----
            ident = cpool.tile([128, 128], BF16)
            ones_bf = cpool.tile([1, 128], BF16)
            ones_f = cpool.tile([1, 128], F32)
            epsc = cpool.tile([128, 1], F32)
            poolind = cpool.tile([128, NGRP, 2, 128], BF16)
            maskt = cpool.tile([128, NGRP], BF16)
            if not NO_CONST:
                make_identity(nc, ident[:, :])
                nc.vector.memset(ones_bf[:, :], 1.0)
                nc.vector.memset(ones_f[:, :], 1.0)
                nc.vector.memset(epsc[:, :], EPS)
                nc.sync.dma_start(out=poolind[:, :, :, :], in_=p_pool[:, :, :, :])
                nc.sync.dma_start(out=maskt[:, :], in_=p_mask[:, :])
            gidx = cpool.tile([128, K * 8], I16)
            nc.sync.dma_start(out=gidx[:, :], in_=p_gidx[:, :])
            wlin = cpool.tile([128, 128], BF16)
            nc.sync.dma_start(out=wlin[:, :], in_=p_wlin[:, :])
            blin = cpool.tile([1, 128], BF16)
            nc.sync.dma_start(out=blin[:, :], in_=p_blin[:, :])

            # ---- projection: x1 = x0 @ lin_W + lin_b -> bf16 table0 ----
            x0T = bigp.tile([128, NPAD], BF16, tag="x0T")
            nc.sync.dma_start(out=x0T[:, :], in_=p_x0T[:, :])
            xnew_bf = bigp.tile([128, NGRP, 128], BF16, tag="xnewbf")
            for t in range(0 if NO_PROJ else NGRP):
                ps = ffnps.tile([128, 128], F32, tag="hps")
                nc.tensor.matmul(ps[:, :], lhsT=x0T[:, t * 128:(t + 1) * 128],
                                 rhs=wlin[:, :], start=True,
                                 stop=flags["blin_zero"])
                if not flags["blin_zero"]:
                    nc.tensor.matmul(ps[:, :], lhsT=ones_bf[:, :],
                                     rhs=blin[:, :], start=False, stop=True)
                nc.scalar.copy(out=xnew_bf[:, t, :], in_=ps[:, :])
            if NO_PROJ:
                nc.sync.dma_start(
                    out=ag_in[:, :].rearrange("(t p) f -> p (t f)", p=128),
                    in_=x0T[:, :],
                )
            else:
                nc.sync.dma_start(
                    out=ag_in[:, :].rearrange("(t p) f -> p t f", p=128),
                    in_=xnew_bf[:, :, :],
                )
            ag0 = nc.gpsimd.collective_compute(
                "AllGather", ALU.bypass, replica_groups=RG,
                ins=[ag_in.ap().opt()], outs=[tables[0].ap().opt()],
            )
            ag_insts = [ag0, None, None]
            if DUMP:
                tcp = nc.sync.dma_start(out=o_dtbl[:, :], in_=tables[0][:, :])
                add_dep_helper(tcp.ins, ag0.ins, sync=True, reason="dbg tbl dump")

            # ---- layers ----
            aggrA = bigp.tile([128, NWIN * W], F32, tag="aggrA")
            y_sb = bigp.tile([128, NGRP, 128], F32, tag="ysb")
            xnew_f = bigp.tile([128, NGRP, 128], F32, tag="xnewf")

            for l in range(NL):
                # Hard inter-layer barrier: the big SBUF buffers (xnew_f,
                # y_sb, aggrA, xnew_bf) are reused across layers and Tile
                # misses some cross-layer WAR edges on HW (sim passes but
                # HW shows element-level races). ~2us each, negligible.
                tc.strict_bb_all_engine_barrier()
                table = tables[l]
                tblA = table[0:BSPLIT, :]
                tblB = table[BSPLIT:TBL, :]

                w1 = ppool.tile([128, 256], BF16, tag="w1")
                nc.sync.dma_start(out=w1[:, :], in_=p_W1[l, :, :])
                w2 = ppool.tile([128, 2, 128], BF16, tag="w2")
                nc.sync.dma_start(out=w2[:, :, :], in_=p_W2[l, :, :, :])
                b1r = ppool.tile([1, 256], BF16, tag="b1r")
                nc.sync.dma_start(out=b1r[:, :], in_=p_b1[l, :, :])
                b2r = ppool.tile([1, 128], BF16, tag="b2r")
                nc.sync.dma_start(out=b2r[:, :], in_=p_b2[l, :, :])
                if not flags["ln_triv"]:
                    lng = ppool.tile([128, 256], F32, tag="lng")
                    nc.sync.dma_start(out=lng[:, :], in_=p_lng[l, :, :])
                    lnb = ppool.tile([128, 256], F32, tag="lnb")
                    nc.sync.dma_start(out=lnb[:, :], in_=p_lnb[l, :, :])
                bng = ppool.tile([1, 128], F32, tag="bng")
                nc.sync.dma_start(out=bng[:, :], in_=p_bng[l, :, :])
                bnb = ppool.tile([1, 128], F32, tag="bnb")
                nc.sync.dma_start(out=bnb[:, :], in_=p_bnb[l, :, :])

                if not NO_LONGPS:
                    stats_ps = longps.tile([1, 256], F32, tag="stats")
                    pool_ps = longps.tile([128, 256], F32, tag="poolps")

                # ---------- seg-matmul passes ----------
                def run_pass(calls, tbl_ap, pass_b, dep_inst):
                    cur_ps = {}

                    def win_ps(w):
                        g = w // WPG
                        if g not in cur_ps:
                            cur_ps[g] = segps.tile([128, 128], F32, tag="seg", name=f"seg{g}")
                        return cur_ps[g], (w % WPG) * W

                    done_in_win = {}
                    for (k0, nch) in calls:
                        gout = goutp.tile([128, SC, 128], BF16, tag="gout")
                        indt = indp.tile([128, SC, W], BF16, tag="ind")
                        if not NO_IND:
                            nc.sync.dma_start(out=indt[:, 0:nch, :],
                                              in_=p_ind[:, k0:k0 + nch, :])
                        g_inst = nc.gpsimd.dma_gather(
                            out_ap=gout[:, 0:nch, :],
                            in_ap=tbl_ap,
                            idxs_ap=gidx[:, k0 * 8:(k0 + nch) * 8],
                            num_idxs=nch * 128,
                            num_idxs_reg=nch * 128,
                            elem_size=128,
                        )
                        if dep_inst is not None:
                            add_dep_helper(g_inst.ins, dep_inst.ins, sync=True,
                                           reason="gather reads AG table")
                        if NO_MM:
                            continue
                        for j in range(nch):
                            k = k0 + j
                            w = int(chunk_win[k])
                            b = 1 if pass_b else 0
                            first = done_in_win.get(w, 0) == 0
                            done_in_win[w] = done_in_win.get(w, 0) + 1
                            last = done_in_win[w] == cpw[b, w]
                            if DVE_MM:
                                sc1 = work.tile([128, 64], F32, tag="dvemm",
                                                name=f"dv{k}{pass_b}")
                                nc.vector.tensor_tensor(
                                    out=sc1[:, :], in0=gout[:, j, 0:64],
                                    in1=indt[:, j, :], op=ALU.add)
                                if last and w % WPG == 1:
                                    yield w // WPG, None
                                continue
                            ps, coff = win_ps(w)
                            # PSUM start=True zeroes the whole 2KB bank, so
                            # only the first matmul touching the tile starts.
                            nc.tensor.matmul(
                                ps[:, coff:coff + W],
                                lhsT=gout[:, j, :],
                                rhs=indt[:, j, :],
                                start=(first and w % WPG == 0),
                                stop=(last and w % WPG == 1),
                                skip_group_check=True,
                            )
                            if last and w % WPG == 1:
                                g = w // WPG
                                yield g, cur_ps.pop(g)

                # pass A: evict into aggrA
                for g, ps in run_pass(calls_A, tblA, False, ag_insts[l]):
                    if ps is None:
                        continue
                    nc.scalar.copy(out=aggrA[:, g * 128:(g + 1) * 128],
                                   in_=ps[:, :])

                if DUMP and l == 0:
                    nc.sync.dma_start(out=o_daggr[:, :], in_=aggrA[:, :])
                # pass B: combine + FFN per group
                for g, psB in run_pass(calls_B, tblB, True, ag_insts[l]):
                    aggrT_f = work.tile([128, 128], F32, tag="aggrTf")
                    if psB is None:
                        nc.vector.memset(aggrT_f[:, :], 0.0)
                    else:
                        nc.vector.tensor_tensor(
                            out=aggrT_f[:, :], in0=psB[:, :],
                            in1=aggrA[:, g * 128:(g + 1) * 128], op=ALU.add)
                    if not DO_FFN:
                        nc.scalar.copy(out=y_sb[:, g, :], in_=aggrT_f[:, :])
                        continue
                    aggrT_bf = work.tile([128, 128], BF16, tag="aggrTbf")
                    nc.scalar.copy(out=aggrT_bf[:, :], in_=aggrT_f[:, :])

                    h_ps = ffnps.tile([128, 256], F32, tag="hps")
                    nc.tensor.matmul(h_ps[:, :], lhsT=aggrT_bf[:, :],
                                     rhs=w1[:, :], start=True,
                                     stop=flags["b1_zero"])
                    if not flags["b1_zero"]:
                        nc.tensor.matmul(h_ps[:, :], lhsT=ones_bf[:, :],
                                         rhs=b1r[:, :], start=False, stop=True)
                    st6 = work.tile([128, 6], F32, tag="st6")
                    nc.vector.bn_stats(st6[:, :], h_ps[:, :])
                    mv = work.tile([128, 2], F32, tag="mv")
                    nc.vector.bn_aggr(mv[:, :], st6[:, :])
                    sd = work.tile([128, 1], F32, tag="sd")
                    nc.scalar.activation(sd[:, :], mv[:, 1:2], AF.Sqrt, bias=epsc[:, :])
                    inv = work.tile([128, 1], F32, tag="inv")
                    nc.vector.reciprocal(inv[:, :], sd[:, :])
                    t_sb = work.tile([128, 256], F32, tag="tsb")
                    nc.vector.tensor_scalar(
                        out=t_sb[:, :], in0=h_ps[:, :],
                        scalar1=mv[:, 0:1], scalar2=inv[:, :],
                        op0=ALU.subtract, op1=ALU.mult)
                    if not flags["ln_triv"]:
                        u = work.tile([128, 256], F32, tag="lnu")
                        nc.vector.tensor_tensor(out=u[:, :], in0=t_sb[:, :],
                                                in1=lng[:, :], op=ALU.mult)
                        nc.vector.tensor_tensor(out=u[:, :], in0=u[:, :],
                                                in1=lnb[:, :], op=ALU.add)
                        relu_src = u
                    else:
                        relu_src = t_sb
                    rh_bf = work.tile([128, 256], BF16, tag="rhbf")
                    nc.scalar.activation(rh_bf[:, :], relu_src[:, :], AF.Relu)

                    tr0 = ffnps.tile([128, 128], BF16, tag="fsm")
                    nc.tensor.transpose(tr0[:, :], rh_bf[:, 0:128], ident[:, :])
                    tr1 = ffnps.tile([128, 128], BF16, tag="fsm")
                    nc.tensor.transpose(tr1[:, :], rh_bf[:, 128:256], ident[:, :])
                    rhT0 = work.tile([128, 128], BF16, tag="rhT0")
                    nc.vector.tensor_copy(out=rhT0[:, :], in_=tr0[:, :])
                    rhT1 = work.tile([128, 128], BF16, tag="rhT1")
                    nc.scalar.copy(out=rhT1[:, :], in_=tr1[:, :])

                    y_ps = ffnps.tile([128, 128], F32, tag="fsm")
                    nc.tensor.matmul(y_ps[:, :], lhsT=rhT0[:, :],
                                     rhs=w2[:, 0, :], start=True, stop=False)
                    nc.tensor.matmul(y_ps[:, :], lhsT=rhT1[:, :],
                                     rhs=w2[:, 1, :], start=False, stop=False)
                    if not flags["b2_zero"]:
                        nc.tensor.matmul(y_ps[:, :], lhsT=ones_bf[:, :],
                                         rhs=b2r[:, :], start=False, stop=False)
                    nc.tensor.matmul(y_ps[:, :], lhsT=aggrT_bf[:, :],
                                     rhs=ident[:, :], start=False, stop=True)

                    nc.scalar.activation(y_sb[:, g, :], y_ps[:, :], AF.Relu)
                    ybf = work.tile([128, 128], BF16, tag="ybf")
                    nc.scalar.copy(out=ybf[:, :], in_=y_sb[:, g, :])
                    ysq = work.tile([128, 128], BF16, tag="ysq")
                    nc.scalar.activation(ysq[:, :], y_sb[:, g, :], AF.Square)
                    nc.tensor.matmul(stats_ps[:, 0:128],
                                     lhsT=maskt[:, g:g + 1], rhs=ybf[:, :],
                                     start=(g == 0), stop=False,
                                     skip_group_check=True)
                    nc.tensor.matmul(stats_ps[:, 128:256],
                                     lhsT=maskt[:, g:g + 1], rhs=ysq[:, :],
                                     start=False, stop=(g == NGRP - 1),
                                     skip_group_check=True)

                if DUMP and l == 0:
                    nc.sync.dma_start(out=o_dysb[:, :, :], in_=y_sb[:, :, :])
                # ---------- BN stats all-reduce ----------
                if not DO_TAIL:
                    nc.sync.dma_start(out=o_xs[l, :, :, :], in_=y_sb[:, :, :])
                    continue
                st_sb = work.tile([1, 256], F32, tag="stsb")
                nc.scalar.copy(out=st_sb[:, :], in_=stats_ps[:, :])
                nc.gpsimd.dma_start(out=ar_ins[l][:, :], in_=st_sb[:, :])
                ar_inst = nc.gpsimd.collective_compute(
                    "AllReduce", ALU.add, replica_groups=RG,
                    ins=[ar_ins[l].ap().opt()], outs=[ar_outs[l].ap().opt()],
                )
                stg = work.tile([1, 256], F32, tag="stg")
                stg_rd = nc.sync.dma_start(out=stg[:, :], in_=ar_outs[l][:, :])
                add_dep_helper(stg_rd.ins, ar_inst.ins, sync=True,
                               reason="stats read after AllReduce")
                st2 = work.tile([1, 256], F32, tag="st2")  # [s | t]
                mrow = work.tile([1, 128], F32, tag="mrow")
                nc.vector.tensor_scalar_mul(mrow[:, :], stg[:, 0:128], 1.0 / N)
                qrow = work.tile([1, 128], F32, tag="qrow")
                nc.vector.tensor_scalar_mul(qrow[:, :], stg[:, 128:256], 1.0 / N)
                msq = work.tile([1, 128], F32, tag="msq")
                nc.vector.tensor_tensor(out=msq[:, :], in0=mrow[:, :],
                                        in1=mrow[:, :], op=ALU.mult)
                vrow = work.tile([1, 128], F32, tag="vrow")
                nc.vector.tensor_tensor(out=vrow[:, :], in0=qrow[:, :],
                                        in1=msq[:, :], op=ALU.subtract)
                sdr = work.tile([1, 128], F32, tag="sdr")
                nc.scalar.activation(sdr[:, :], vrow[:, :], AF.Sqrt, bias=epsc[0:1, :])
                invr = work.tile([1, 128], F32, tag="invr")
                nc.vector.reciprocal(invr[:, :], sdr[:, :])
                if flags["bng_one"]:
                    nc.vector.tensor_copy(out=st2[:, 0:128], in_=invr[:, :])
                else:
                    nc.vector.tensor_tensor(out=st2[:, 0:128], in0=invr[:, :],
                                            in1=bng[:, :], op=ALU.mult)
                ms = work.tile([1, 128], F32, tag="ms")
                nc.vector.tensor_tensor(out=ms[:, :], in0=mrow[:, :],
                                        in1=st2[:, 0:128], op=ALU.mult)
                if flags["bnb_zero"]:
                    nc.vector.tensor_scalar_mul(st2[:, 128:256], ms[:, :], -1.0)
                else:
                    nc.vector.tensor_tensor(out=st2[:, 128:256], in0=bnb[:, :],
                                            in1=ms[:, :], op=ALU.subtract)
                bc_ps = ffnps.tile([128, 256], F32, tag="hps")
                nc.tensor.matmul(bc_ps[:, :], lhsT=ones_f[:, :], rhs=st2[:, :],
                                 start=True, stop=True)
                st_bc = work.tile([128, 256], F32, tag="stbc")
                nc.scalar.copy(out=st_bc[:, :], in_=bc_ps[:, :])

                # ---------- BN apply + pool ----------
                for g in range(NGRP):
                    tmp = work.tile([128, 128], F32, tag="bntmp")
                    nc.vector.tensor_tensor(out=tmp[:, :], in0=y_sb[:, g, :],
                                            in1=st_bc[:, 0:128], op=ALU.mult)
                    nc.vector.tensor_tensor(out=xnew_f[:, g, :], in0=tmp[:, :],
                                            in1=st_bc[:, 128:256], op=ALU.add)
                    nc.scalar.copy(out=xnew_bf[:, g, :], in_=xnew_f[:, g, :])
                    nc.tensor.matmul(pool_ps[:, 0:128], lhsT=poolind[:, g, 0, :],
                                     rhs=xnew_bf[:, g, :], start=(g == 0),
                                     stop=False, skip_group_check=True)
                    nc.tensor.matmul(pool_ps[:, 128:256], lhsT=poolind[:, g, 1, :],
                                     rhs=xnew_bf[:, g, :], start=False,
                                     stop=(g == NGRP - 1), skip_group_check=True)

                nc.sync.dma_start(out=o_xs[l, :, :, :], in_=xnew_f[:, :, :])
                pl0 = work.tile([128, 128], F32, tag="pl0")
                nc.scalar.copy(out=pl0[:, :], in_=pool_ps[:, 0:128])
                nc.sync.dma_start(out=o_pool[l, 0, :, :], in_=pl0[:, :])
                pl1 = work.tile([128, 128], F32, tag="pl1")
                nc.scalar.copy(out=pl1[:, :], in_=pool_ps[:, 128:256])
                nc.sync.dma_start(out=o_pool[l, 1, :, :], in_=pl1[:, :])

                if l < L - 1:
                    nc.sync.dma_start(
                        out=ag_in[:, :].rearrange("(t p) f -> p t f", p=128),
                        in_=xnew_bf[:, :, :],
                    )
                    ag_insts[l + 1] = nc.gpsimd.collective_compute(
                        "AllGather", ALU.bypass, replica_groups=RG,
                        ins=[ag_in.ap().opt()],
                        outs=[tables[l + 1].ap().opt()],
                    )

    nc.compile()
    return nc


def kernel(**inputs):
    x = _to_np(inputs["x"], np.float32)
    ei = _to_np(inputs["edge_index"], np.int64)
    batch = _to_np(inputs["batch"], np.int64)
    ew = _to_np(inputs["edge_weight"], np.float32)
    lin_W = _to_np(inputs["lin_W"], np.float32)
    lin_b = _to_np(inputs["lin_b"], np.float32)
    W1 = _to_np(inputs["W1"], np.float32)
    b1 = _to_np(inputs["b1"], np.float32)
    ln_gamma = _to_np(inputs["ln_gamma"], np.float32)
    ln_beta = _to_np(inputs["ln_beta"], np.float32)
    W2 = _to_np(inputs["W2"], np.float32)
    b2 = _to_np(inputs["b2"], np.float32)
    bn_gamma = _to_np(inputs["bn_gamma"], np.float32)
    bn_beta = _to_np(inputs["bn_beta"], np.float32)

    src, dst = ei[0], ei[1]
    plan, percore = _prep(x, src, dst, ew, batch)

    flags = dict(
        blin_zero=bool(np.all(lin_b == 0)),
        b1_zero=bool(np.all(b1 == 0)),
        b2_zero=bool(np.all(b2 == 0)),
        ln_triv=bool(np.all(ln_gamma == 1) and np.all(ln_beta == 0)),
        bng_one=bool(np.all(bn_gamma == 1)),
        bnb_zero=bool(np.all(bn_beta == 0)),
    )

    nc = _build(plan, flags)

    shared = dict(
        wlin=lin_W.astype(NP_BF16),
        blin=lin_b.reshape(1, 128).astype(NP_BF16),
        W1=W1.astype(NP_BF16),
        b1=b1.reshape(L, 1, 256).astype(NP_BF16),
        W2=W2.reshape(L, 2, 128, 128).transpose(0, 2, 1, 3).copy().astype(NP_BF16),
        b2=b2.reshape(L, 1, 128).astype(NP_BF16),
        lng=np.broadcast_to(ln_gamma.reshape(L, 1, 256),
                            (L, 128, 256)).copy().astype(np.float32),
        lnb=np.broadcast_to(ln_beta.reshape(L, 1, 256),
                            (L, 128, 256)).copy().astype(np.float32),
        bng=bn_gamma.reshape(L, 1, 128).astype(np.float32),
        bnb=bn_beta.reshape(L, 1, 128).astype(np.float32),
    )
    in_maps = []
    for c in range(C):
        m = dict(shared)
        m["x0T"] = percore["x0T"][c]
        m["gidx"] = percore["gidx"][c]
        m["ind"] = percore["ind"][c]
        m["poolind"] = percore["pool"][c]
        m["mask"] = percore["mask"][c]
        in_maps.append(m)

    res = run_bass_kernel_spmd(nc, in_maps, core_ids=list(range(C)))
    global _last_run
    _last_run = res

    xs = np.zeros((N, L * DIM), np.float32)
    pooled = np.zeros((G, L * DIM), np.float32)
    for c in range(C):
        r = res.results[c]
        xo = r["xs_out"]       # [L, 128, NGRP, 128]
        po = r["pool_out"]     # [L, 2, 128, 128]
        for l in range(L):
            sl = xo[l].transpose(1, 0, 2).reshape(NPAD, 128)[:NPC]
            xs[c * NPC:(c + 1) * NPC, l * DIM:(l + 1) * DIM] = sl
            pooled[:, l * DIM:(l + 1) * DIM] += po[l].reshape(256, 128)
    return pooled, xs


# revision 23
# speedup vs baseline: 1.1752x; 1.1752x over previous
"""GIN-style GNN encoder (3-layer message passing + FFN + norms + segment pool)
on 8 Trainium2 NeuronCores.

Strategy:
- Nodes are range-partitioned across the 8 cores (5000 each, padded to 5120).
  Edges are assigned to the core owning their *destination* node, so the
  scatter-add is core-local (no all-reduce over node features).
- Per core, local nodes are processed in 40 groups of 128; each group has two
  64-node windows. Edges are bucketed by (src-table-row < 32768) so gather
  indices fit int16, grouped by window, and padded to 128-edge chunks. The
  chunk counts per (bucket, window) are maxed over cores so all 8 cores run
  one identical SPMD program (pad chunks gather row 0 with all-zero
  indicators).
- Gather: dma_gather from a replicated bf16 node table in DRAM (the AllGather
  output of the previous layer). Scatter-add: per chunk, matmul
  (messages[e,f] as stationary)T @ indicator[e,64nodes] accumulating
  aggr^T in PSUM; the indicator carries the edge weight.
- FFN/LayerNorm run with nodes on partitions; BatchNorm statistics are
  one AllReduce of [1,256] per layer; pooling is an indicator matmul into
  per-core partials that the host sums.
"""

import os
import numpy as np
import ml_dtypes

import concourse.bass as bass
import concourse.bacc as bacc
import concourse.mybir as mybir
from concourse.bass_utils import run_bass_kernel_spmd
from concourse.tile import TileContext
from concourse.masks import make_identity
from concourse.tile_rust import add_dep_helper

# Problem constants (hardcoded per harness contract)
N = 40000
E = 640000
F_IN = 128
DIM = 128
L = 3
G = 256
EPS = 1e-5

C = 8            # cores
NPC = 5000       # real nodes per core
NGRP = 40        # groups of 128 (includes padding)
NPAD = NGRP * 128  # 5120
TBL = C * NPAD   # 40960 table rows
BSPLIT = 32768   # int16 bucket split in table-row space
W = 64           # indicator window (nodes)
WPG = 2          # windows per group
NWIN = NGRP * WPG  # 80 windows/core
SC = 8   # chunks per gather super-call (1024 idxs; >=2048 overflows the SWDGE ring and hangs)

BF16 = mybir.dt.bfloat16
F32 = mybir.dt.float32
I16 = mybir.dt.int16

AF = mybir.ActivationFunctionType
ALU = mybir.AluOpType

NP_BF16 = ml_dtypes.bfloat16

_last_run = None  # BassKernelResults of the most recent kernel() call


def _to_np(a, dtype=None):
    a = np.asarray(a)
    if dtype is not None:
        a = a.astype(dtype)
    return a


def _prep(x, src, dst, ew, batch):
    """Host-side sharding. Returns (plan, per-core arrays)."""
    core = dst // NPC
    ldst = dst - core * NPC
    win = ldst // W
    wcol = ldst - win * W
    trow = (src // NPC) * NPAD + (src % NPC)
    bucket = (trow >= BSPLIT).astype(np.int64)

    gid = (core * 2 + bucket) * NWIN + win
    cnt = np.bincount(gid, minlength=C * 2 * NWIN).reshape(C, 2, NWIN)
    cpw = np.maximum(np.ceil(cnt / 128.0).astype(np.int64).max(axis=0), 1)  # [2, NWIN]
    KA = int(cpw[0].sum())
    KB = int(cpw[1].sum())
    K = KA + KB
    chunk_base = np.zeros((2, NWIN), np.int64)
    chunk_base[0, :] = np.concatenate([[0], np.cumsum(cpw[0])[:-1]])
    chunk_base[1, :] = KA + np.concatenate([[0], np.cumsum(cpw[1])[:-1]])

    # per-edge position within its (core,bucket,win) run
    order = np.argsort(gid, kind="stable")
    sg = gid[order]
    runstart = np.zeros(len(sg), np.int64)
    newrun = np.ones(len(sg), bool)
    newrun[1:] = sg[1:] != sg[:-1]
    runidx = np.flatnonzero(newrun)
    runstart[runidx] = runidx
    runstart = np.maximum.accumulate(runstart)
    pos_sorted = np.arange(len(sg)) - runstart
    pos = np.empty(len(sg), np.int64)
    pos[order] = pos_sorted

    chunk = chunk_base[bucket, win] + pos // 128
    prow = pos % 128

    gidx_all = []
    ind_all = []
    for c in range(C):
        m = core == c
        lin = np.zeros(K * 128, np.int32)
        lin[chunk[m] * 128 + prow[m]] = trow[m] - bucket[m] * BSPLIT
        assert lin.max() < 32768 and lin.min() >= 0
        # wrap for dma_gather: index i -> partition i%16, col i//16; replicate x8
        arr16 = lin.astype(np.int16).reshape(K * 8, 16).T  # [16, K*8]
        gidx_all.append(np.tile(arr16, (8, 1)))            # [128, K*8]
        ind = np.zeros((128, K, W), NP_BF16)
        ind[prow[m], chunk[m], wcol[m]] = ew[m].astype(NP_BF16)
        ind_all.append(ind)

    # pooling indicators + node mask
    pool_all = []
    mask_all = []
    for c in range(C):
        pi = np.zeros((128, NGRP, 2, 128), NP_BF16)
        b_loc = batch[c * NPC:(c + 1) * NPC]
        nl = np.arange(NPC)
        t = nl // 128
        p = nl % 128
        pi[p, t, b_loc // 128, b_loc % 128] = 1.0
        pool_all.append(pi)
        mk = np.zeros((128, NGRP), NP_BF16)
        mk[p, t] = 1.0
        mask_all.append(mk)

    # x transposed slices, bf16
    x0T_all = []
    for c in range(C):
        s = np.zeros((128, NPAD), NP_BF16)
        xs = x[c * NPC:(c + 1) * NPC]  # [5000, F_IN]
        # node local n = t*128 + p -> column t*128+p? layout [p, t, f] flattened as
        # [128, NGRP*128] with column t*128+f. We store as [f? no]:
        # lhsT for proj needs [f, node] per tile: columns = node within tile.
        # Use layout [128 f, NPAD nodes]: s[f, n] = x[n, f]
        s[:, :NPC] = xs.T.astype(NP_BF16)
        x0T_all.append(s)

    plan = dict(cpw=cpw, KA=KA, KB=KB, K=K, chunk_base=chunk_base)
    percore = dict(gidx=gidx_all, ind=ind_all, pool=pool_all, mask=mask_all,
                   x0T=x0T_all)
    return plan, percore


def _build(plan, flags, dbg=None):
    """Build the SPMD Bass program (identical across cores)."""
    cpw = plan["cpw"]
    KA, KB, K = plan["KA"], plan["KB"], plan["K"]
    if dbg is None:
        dbg = {}
    NL = dbg.get("nl", L)          # layers to actually run
    DO_FFN = dbg.get("ffn", True)  # run FFN block per group
    DO_TAIL = dbg.get("tail", True)  # stats AR + BN apply + pool + AG
    NO_MM = dbg.get("nomm", False)   # skip seg matmuls/evictions
    NO_IND = dbg.get("noind", False)  # skip indicator DMAs
    NO_CONST = dbg.get("noconst", False)  # skip ident/poolind/mask/eps/ones loads
    NO_PROJ = dbg.get("noproj", False)  # skip proj matmuls (raw x -> table)
    NO_LONGPS = dbg.get("nolongps", False)  # skip stats/pool psum allocation
    DVE_MM = dbg.get("dvemm", False)  # replace seg MMs with DVE consumes
    DUMP = dbg.get("dump", False)    # dump stage intermediates (layer 0)

    nc = bacc.Bacc("TRN2", target_bir_lowering=False, num_swdge_queues=4)

    # ---- DRAM parameters ----
    p_x0T = nc.declare_dram_parameter("x0T", [128, NPAD], BF16, isOutput=False)
    p_gidx = nc.declare_dram_parameter("gidx", [128, K * 8], I16, isOutput=False)
    p_ind = nc.declare_dram_parameter("ind", [128, K, W], BF16, isOutput=False)
    p_pool = nc.declare_dram_parameter("poolind", [128, NGRP, 2, 128], BF16,
                                       isOutput=False)
    p_mask = nc.declare_dram_parameter("mask", [128, NGRP], BF16, isOutput=False)
    p_wlin = nc.declare_dram_parameter("wlin", [128, 128], BF16, isOutput=False)
    p_blin = nc.declare_dram_parameter("blin", [1, 128], BF16, isOutput=False)
    p_W1 = nc.declare_dram_parameter("W1", [L, 128, 256], BF16, isOutput=False)
    p_b1 = nc.declare_dram_parameter("b1", [L, 1, 256], BF16, isOutput=False)
    # W2 stored as [L, 128, 2, 128]: [:, p, k, :] = W2[l, k*128+p, :]
    p_W2 = nc.declare_dram_parameter("W2", [L, 128, 2, 128], BF16, isOutput=False)
    p_b2 = nc.declare_dram_parameter("b2", [L, 1, 128], BF16, isOutput=False)
    p_lng = nc.declare_dram_parameter("lng", [L, 128, 256], F32, isOutput=False)
    p_lnb = nc.declare_dram_parameter("lnb", [L, 128, 256], F32, isOutput=False)
    p_bng = nc.declare_dram_parameter("bng", [L, 1, 128], F32, isOutput=False)
    p_bnb = nc.declare_dram_parameter("bnb", [L, 1, 128], F32, isOutput=False)
    o_xs = nc.declare_dram_parameter("xs_out", [L, 128, NGRP, 128], F32,
                                     isOutput=True)
    o_pool = nc.declare_dram_parameter("pool_out", [L, 2, 128, 128], F32,
                                       isOutput=True)
    if dbg and dbg.get("dump", False):
        o_dtbl = nc.declare_dram_parameter("dbg_tbl", [TBL, 128], BF16,
                                           isOutput=True)
        o_daggr = nc.declare_dram_parameter("dbg_aggrA", [128, NWIN * W], F32,
                                            isOutput=True)
        o_dysb = nc.declare_dram_parameter("dbg_ysb", [128, NGRP, 128], F32,
                                           isOutput=True)

    # internal DRAM
    ag_in = nc.dram_tensor("ag_in", [NPAD, 128], BF16)
    tables = [nc.dram_tensor(f"table{l}", [TBL, 128], BF16, addr_space="Shared")
              for l in range(L)]
    ar_ins = [nc.dram_tensor(f"ar_in{l}", [1, 256], F32) for l in range(L)]
    ar_outs = [nc.dram_tensor(f"ar_out{l}", [1, 256], F32, addr_space="Shared")
               for l in range(L)]

    RG = [list(range(C))]

    # chunk metadata: list over k of (bucket, window)
    chunk_win = np.zeros(K, np.int64)
    for b in range(2):
        for w in range(NWIN):
            base = plan["chunk_base"][b, w]
            chunk_win[base:base + cpw[b, w]] = w
    # supercalls within each bucket
    def supercalls(lo, hi):
        out = []
        k = lo
        while k < hi:
            out.append((k, min(SC, hi - k)))
            k += SC
        return out

    calls_A = supercalls(0, KA)
    calls_B = supercalls(KA, K)

    with TileContext(nc) as tc:
        with (
            tc.tile_pool(name="const", bufs=1) as cpool,
            tc.tile_pool(name="params", bufs=2) as ppool,
            tc.tile_pool(name="indp", bufs=2) as indp,
            tc.tile_pool(name="goutp", bufs=2) as goutp,
            tc.tile_pool(name="segps", bufs=2, space="PSUM") as segps,
            tc.tile_pool(name="ffnps", bufs=2, space="PSUM") as ffnps,
            tc.tile_pool(name="longps", bufs=1, space="PSUM") as longps,
            tc.tile_pool(name="work", bufs=3) as work,
            tc.tile_pool(name="bigbuf", bufs=1) as bigp,
        ):
            # ---- constants ----
            ident = cpool.tile([128, 128], BF16)
            ones_bf = cpool.tile([1, 128], BF16)
            ones_f = cpool.tile([1, 128], F32)
            epsc = cpool.tile([128, 1], F32)
            poolind = cpool.tile([128, NGRP, 2, 128], BF16)
            maskt = cpool.tile([128, NGRP], BF16)
            if not NO_CONST:
                make_identity(nc, ident[:, :])
                nc.vector.memset(ones_bf[:, :], 1.0)
                nc.vector.memset(ones_f[:, :], 1.0)
                nc.vector.memset(epsc[:, :], EPS)
                nc.sync.dma_start(out=poolind[:, :, :, :], in_=p_pool[:, :, :, :])
                nc.sync.dma_start(out=maskt[:, :], in_=p_mask[:, :])
            gidx = cpool.tile([128, K * 8], I16)
            nc.sync.dma_start(out=gidx[:, :], in_=p_gidx[:, :])
            wlin = cpool.tile([128, 128], BF16)
            nc.sync.dma_start(out=wlin[:, :], in_=p_wlin[:, :])
            blin = cpool.tile([1, 128], BF16)
            nc.sync.dma_start(out=blin[:, :], in_=p_blin[:, :])

            # ---- projection: x1 = x0 @ lin_W + lin_b -> bf16 table0 ----
            x0T = bigp.tile([128, NPAD], BF16, tag="x0T")
            nc.sync.dma_start(out=x0T[:, :], in_=p_x0T[:, :])
            xnew_bf = bigp.tile([128, NGRP, 128], BF16, tag="xnewbf")
            for t in range(0 if NO_PROJ else NGRP):
                ps = ffnps.tile([128, 128], F32, tag="hps")
                nc.tensor.matmul(ps[:, :], lhsT=x0T[:, t * 128:(t + 1) * 128],
                                 rhs=wlin[:, :], start=True,
                                 stop=flags["blin_zero"])
                if not flags["blin_zero"]:
                    nc.tensor.matmul(ps[:, :], lhsT=ones_bf[:, :],
                                     rhs=blin[:, :], start=False, stop=True)
                nc.scalar.copy(out=xnew_bf[:, t, :], in_=ps[:, :])
            if NO_PROJ:
                nc.sync.dma_start(
                    out=ag_in[:, :].rearrange("(t p) f -> p (t f)", p=128),
                    in_=x0T[:, :],
                )
            else:
                nc.sync.dma_start(
                    out=ag_in[:, :].rearrange("(t p) f -> p t f", p=128),
                    in_=xnew_bf[:, :, :],
                )
            ag0 = nc.gpsimd.collective_compute(
                "AllGather", ALU.bypass, replica_groups=RG,
                ins=[ag_in.ap().opt()], outs=[tables[0].ap().opt()],
            )
            ag_insts = [ag0, None, None]
            if DUMP:
                tcp = nc.sync.dma_start(out=o_dtbl[:, :], in_=tables[0][:, :])
                add_dep_helper(tcp.ins, ag0.ins, sync=True, reason="dbg tbl dump")

            # ---- layers ----
            aggrA = bigp.tile([128, NWIN * W], F32, tag="aggrA")
            y_sb = bigp.tile([128, NGRP, 128], F32, tag="ysb")
            xnew_f = bigp.tile([128, NGRP, 128], F32, tag="xnewf")

            for l in range(NL):
                # Hard inter-layer barrier: the big SBUF buffers (xnew_f,
                # y_sb, aggrA, xnew_bf) are reused across layers and Tile
                # misses some cross-layer WAR edges on HW (sim passes but
                # HW shows element-level races). ~2us each, negligible.
                tc.strict_bb_all_engine_barrier()
                table = tables[l]
                tblA = table[0:BSPLIT, :]
                tblB = table[BSPLIT:TBL, :]

                w1 = ppool.tile([128, 256], BF16, tag="w1")
                nc.sync.dma_start(out=w1[:, :], in_=p_W1[l, :, :])
                w2 = ppool.tile([128, 2, 128], BF16, tag="w2")
                nc.sync.dma_start(out=w2[:, :, :], in_=p_W2[l, :, :, :])
                b1r = ppool.tile([1, 256], BF16, tag="b1r")
                nc.sync.dma_start(out=b1r[:, :], in_=p_b1[l, :, :])
                b2r = ppool.tile([1, 128], BF16, tag="b2r")
                nc.sync.dma_start(out=b2r[:, :], in_=p_b2[l, :, :])
                if not flags["ln_triv"]:
                    lng = ppool.tile([128, 256], F32, tag="lng")
                    nc.sync.dma_start(out=lng[:, :], in_=p_lng[l, :, :])
                    lnb = ppool.tile([128, 256], F32, tag="lnb")
                    nc.sync.dma_start(out=lnb[:, :], in_=p_lnb[l, :, :])
                bng = ppool.tile([1, 128], F32, tag="bng")
                nc.sync.dma_start(out=bng[:, :], in_=p_bng[l, :, :])
                bnb = ppool.tile([1, 128], F32, tag="bnb")
                nc.sync.dma_start(out=bnb[:, :], in_=p_bnb[l, :, :])

                if not NO_LONGPS:
                    stats_ps = longps.tile([1, 256], F32, tag="stats")
                    pool_ps = longps.tile([128, 256], F32, tag="poolps")

                # ---------- seg-matmul passes ----------
                def run_pass(calls, tbl_ap, pass_b, dep_inst):
                    cur_ps = {}

                    def win_ps(w):
                        g = w // WPG
                        if g not in cur_ps:
                            cur_ps[g] = segps.tile([128, 128], F32, tag="seg", name=f"seg{g}")
                        return cur_ps[g], (w % WPG) * W

                    done_in_win = {}
                    for (k0, nch) in calls:
                        gout = goutp.tile([128, SC, 128], BF16, tag="gout")
                        indt = indp.tile([128, SC, W], BF16, tag="ind")
                        if not NO_IND:
                            nc.sync.dma_start(out=indt[:, 0:nch, :],
                                              in_=p_ind[:, k0:k0 + nch, :])
                        g_inst = nc.gpsimd.dma_gather(
                            out_ap=gout[:, 0:nch, :],
                            in_ap=tbl_ap,
                            idxs_ap=gidx[:, k0 * 8:(k0 + nch) * 8],
                            num_idxs=nch * 128,
                            num_idxs_reg=nch * 128,
                            elem_size=128,
                            queue_num=(k0 // SC) % 4,
                        )
                        if dep_inst is not None:
                            add_dep_helper(g_inst.ins, dep_inst.ins, sync=True,
                                           reason="gather reads AG table")
                        if NO_MM:
                            continue
                        for j in range(nch):
                            k = k0 + j
                            w = int(chunk_win[k])
                            b = 1 if pass_b else 0
                            first = done_in_win.get(w, 0) == 0
                            done_in_win[w] = done_in_win.get(w, 0) + 1
                            last = done_in_win[w] == cpw[b, w]
                            if DVE_MM:
                                sc1 = work.tile([128, 64], F32, tag="dvemm",
                                                name=f"dv{k}{pass_b}")
                                nc.vector.tensor_tensor(
                                    out=sc1[:, :], in0=gout[:, j, 0:64],
                                    in1=indt[:, j, :], op=ALU.add)
                                if last and w % WPG == 1:
                                    yield w // WPG, None
                                continue
                            ps, coff = win_ps(w)
                            # PSUM start=True zeroes the whole 2KB bank, so
                            # only the first matmul touching the tile starts.
                            nc.tensor.matmul(
                                ps[:, coff:coff + W],
                                lhsT=gout[:, j, :],
                                rhs=indt[:, j, :],
                                start=(first and w % WPG == 0),
                                stop=(last and w % WPG == 1),
                                skip_group_check=True,
                            )
                            if last and w % WPG == 1:
                                g = w // WPG
                                yield g, cur_ps.pop(g)

                # pass A: evict into aggrA
                for g, ps in run_pass(calls_A, tblA, False, ag_insts[l]):
                    if ps is None:
                        continue
                    nc.scalar.copy(out=aggrA[:, g * 128:(g + 1) * 128],
                                   in_=ps[:, :])

                if DUMP and l == 0:
                    nc.sync.dma_start(out=o_daggr[:, :], in_=aggrA[:, :])
                # pass B: combine + FFN per group
                for g, psB in run_pass(calls_B, tblB, True, ag_insts[l]):
                    aggrT_f = work.tile([128, 128], F32, tag="aggrTf")
                    if psB is None:
                        nc.vector.memset(aggrT_f[:, :], 0.0)
                    else:
                        nc.vector.tensor_tensor(
                            out=aggrT_f[:, :], in0=psB[:, :],
                            in1=aggrA[:, g * 128:(g + 1) * 128], op=ALU.add)
                    if not DO_FFN:
                        nc.scalar.copy(out=y_sb[:, g, :], in_=aggrT_f[:, :])
                        continue
                    aggrT_bf = work.tile([128, 128], BF16, tag="aggrTbf")
                    nc.scalar.copy(out=aggrT_bf[:, :], in_=aggrT_f[:, :])

                    h_ps = ffnps.tile([128, 256], F32, tag="hps")
                    nc.tensor.matmul(h_ps[:, :], lhsT=aggrT_bf[:, :],
                                     rhs=w1[:, :], start=True,
                                     stop=flags["b1_zero"])
                    if not flags["b1_zero"]:
                        nc.tensor.matmul(h_ps[:, :], lhsT=ones_bf[:, :],
                                         rhs=b1r[:, :], start=False, stop=True)
                    st6 = work.tile([128, 6], F32, tag="st6")
                    nc.vector.bn_stats(st6[:, :], h_ps[:, :])
                    mv = work.tile([128, 2], F32, tag="mv")
                    nc.vector.bn_aggr(mv[:, :], st6[:, :])
                    sd = work.tile([128, 1], F32, tag="sd")
                    nc.scalar.activation(sd[:, :], mv[:, 1:2], AF.Sqrt, bias=epsc[:, :])
                    inv = work.tile([128, 1], F32, tag="inv")
                    nc.vector.reciprocal(inv[:, :], sd[:, :])
                    t_sb = work.tile([128, 256], F32, tag="tsb")
                    nc.vector.tensor_scalar(
                        out=t_sb[:, :], in0=h_ps[:, :],
                        scalar1=mv[:, 0:1], scalar2=inv[:, :],
                        op0=ALU.subtract, op1=ALU.mult)
                    if not flags["ln_triv"]:
                        u = work.tile([128, 256], F32, tag="lnu")
                        nc.vector.tensor_tensor(out=u[:, :], in0=t_sb[:, :],
                                                in1=lng[:, :], op=ALU.mult)
                        nc.vector.tensor_tensor(out=u[:, :], in0=u[:, :],
                                                in1=lnb[:, :], op=ALU.add)
                        relu_src = u
                    else:
                        relu_src = t_sb
                    rh_bf = work.tile([128, 256], BF16, tag="rhbf")
                    nc.scalar.activation(rh_bf[:, :], relu_src[:, :], AF.Relu)

                    tr0 = ffnps.tile([128, 128], BF16, tag="fsm")
                    nc.tensor.transpose(tr0[:, :], rh_bf[:, 0:128], ident[:, :])
                    tr1 = ffnps.tile([128, 128], BF16, tag="fsm")
                    nc.tensor.transpose(tr1[:, :], rh_bf[:, 128:256], ident[:, :])
                    rhT0 = work.tile([128, 128], BF16, tag="rhT0")
                    nc.vector.tensor_copy(out=rhT0[:, :], in_=tr0[:, :])
                    rhT1 = work.tile([128, 128], BF16, tag="rhT1")
                    nc.scalar.copy(out=rhT1[:, :], in_=tr1[:, :])

                    y_ps = ffnps.tile([128, 128], F32, tag="fsm")
                    nc.tensor.matmul(y_ps[:, :], lhsT=rhT0[:, :],
                                     rhs=w2[:, 0, :], start=True, stop=False)
                    nc.tensor.matmul(y_ps[:, :], lhsT=rhT1[:, :],
                                     rhs=w2[:, 1, :], start=False, stop=False)
                    if not flags["b2_zero"]:
                        nc.tensor.matmul(y_ps[:, :], lhsT=ones_bf[:, :],
                                         rhs=b2r[:, :], start=False, stop=False)
                    nc.tensor.matmul(y_ps[:, :], lhsT=aggrT_bf[:, :],
                                     rhs=ident[:, :], start=False, stop=True)

                    nc.scalar.activation(y_sb[:, g, :], y_ps[:, :], AF.Relu)
                    ybf = work.tile([128, 128], BF16, tag="ybf")
                    nc.scalar.copy(out=ybf[:, :], in_=y_sb[:, g, :])
                    ysq = work.tile([128, 128], BF16, tag="ysq")
                    nc.scalar.activation(ysq[:, :], y_sb[:, g, :], AF.Square)
                    nc.tensor.matmul(stats_ps[:, 0:128],
                                     lhsT=maskt[:, g:g + 1], rhs=ybf[:, :],
                                     start=(g == 0), stop=False,
                                     skip_group_check=True)
                    nc.tensor.matmul(stats_ps[:, 128:256],
                                     lhsT=maskt[:, g:g + 1], rhs=ysq[:, :],
                                     start=False, stop=(g == NGRP - 1),
                                     skip_group_check=True)

                if DUMP and l == 0:
                    nc.sync.dma_start(out=o_dysb[:, :, :], in_=y_sb[:, :, :])
                # ---------- BN stats all-reduce ----------
                if not DO_TAIL:
                    nc.sync.dma_start(out=o_xs[l, :, :, :], in_=y_sb[:, :, :])
                    continue
                st_sb = work.tile([1, 256], F32, tag="stsb")
                nc.scalar.copy(out=st_sb[:, :], in_=stats_ps[:, :])
                nc.gpsimd.dma_start(out=ar_ins[l][:, :], in_=st_sb[:, :])
                ar_inst = nc.gpsimd.collective_compute(
                    "AllReduce", ALU.add, replica_groups=RG,
                    ins=[ar_ins[l].ap().opt()], outs=[ar_outs[l].ap().opt()],
                )
                stg = work.tile([1, 256], F32, tag="stg")
                stg_rd = nc.sync.dma_start(out=stg[:, :], in_=ar_outs[l][:, :])
                add_dep_helper(stg_rd.ins, ar_inst.ins, sync=True,
                               reason="stats read after AllReduce")
                st2 = work.tile([1, 256], F32, tag="st2")  # [s | t]
                mrow = work.tile([1, 128], F32, tag="mrow")
                nc.vector.tensor_scalar_mul(mrow[:, :], stg[:, 0:128], 1.0 / N)
                qrow = work.tile([1, 128], F32, tag="qrow")
                nc.vector.tensor_scalar_mul(qrow[:, :], stg[:, 128:256], 1.0 / N)
                msq = work.tile([1, 128], F32, tag="msq")
                nc.vector.tensor_tensor(out=msq[:, :], in0=mrow[:, :],
                                        in1=mrow[:, :], op=ALU.mult)
                vrow = work.tile([1, 128], F32, tag="vrow")
                nc.vector.tensor_tensor(out=vrow[:, :], in0=qrow[:, :],
                                        in1=msq[:, :], op=ALU.subtract)
                sdr = work.tile([1, 128], F32, tag="sdr")
                nc.scalar.activation(sdr[:, :], vrow[:, :], AF.Sqrt, bias=epsc[0:1, :])
                invr = work.tile([1, 128], F32, tag="invr")
                nc.vector.reciprocal(invr[:, :], sdr[:, :])
                if flags["bng_one"]:
                    nc.vector.tensor_copy(out=st2[:, 0:128], in_=invr[:, :])
                else:
                    nc.vector.tensor_tensor(out=st2[:, 0:128], in0=invr[:, :],
                                            in1=bng[:, :], op=ALU.mult)
                ms = work.tile([1, 128], F32, tag="ms")
                nc.vector.tensor_tensor(out=ms[:, :], in0=mrow[:, :],
                                        in1=st2[:, 0:128], op=ALU.mult)
                if flags["bnb_zero"]:
                    nc.vector.tensor_scalar_mul(st2[:, 128:256], ms[:, :], -1.0)
                else:
                    nc.vector.tensor_tensor(out=st2[:, 128:256], in0=bnb[:, :],
                                            in1=ms[:, :], op=ALU.subtract)
                bc_ps = ffnps.tile([128, 256], F32, tag="hps")
                nc.tensor.matmul(bc_ps[:, :], lhsT=ones_f[:, :], rhs=st2[:, :],
                                 start=True, stop=True)
                st_bc = work.tile([128, 256], F32, tag="stbc")
                nc.scalar.copy(out=st_bc[:, :], in_=bc_ps[:, :])

                # ---------- BN apply + pool ----------
                for g in range(NGRP):
                    tmp = work.tile([128, 128], F32, tag="bntmp")
                    nc.vector.tensor_tensor(out=tmp[:, :], in0=y_sb[:, g, :],
                                            in1=st_bc[:, 0:128], op=ALU.mult)
                    nc.vector.tensor_tensor(out=xnew_f[:, g, :], in0=tmp[:, :],
                                            in1=st_bc[:, 128:256], op=ALU.add)
                    nc.scalar.copy(out=xnew_bf[:, g, :], in_=xnew_f[:, g, :])
                    nc.tensor.matmul(pool_ps[:, 0:128], lhsT=poolind[:, g, 0, :],
                                     rhs=xnew_bf[:, g, :], start=(g == 0),
                                     stop=False, skip_group_check=True)
                    nc.tensor.matmul(pool_ps[:, 128:256], lhsT=poolind[:, g, 1, :],
                                     rhs=xnew_bf[:, g, :], start=False,
                                     stop=(g == NGRP - 1), skip_group_check=True)

                nc.sync.dma_start(out=o_xs[l, :, :, :], in_=xnew_f[:, :, :])
                pl0 = work.tile([128, 128], F32, tag="pl0")
                nc.scalar.copy(out=pl0[:, :], in_=pool_ps[:, 0:128])
                nc.sync.dma_start(out=o_pool[l, 0, :, :], in_=pl0[:, :])
                pl1 = work.tile([128, 128], F32, tag="pl1")
                nc.scalar.copy(out=pl1[:, :], in_=pool_ps[:, 128:256])
                nc.sync.dma_start(out=o_pool[l, 1, :, :], in_=pl1[:, :])

                if l < L - 1:
                    nc.sync.dma_start(
                        out=ag_in[:, :].rearrange("(t p) f -> p t f", p=128),
                        in_=xnew_bf[:, :, :],
                    )
                    ag_insts[l + 1] = nc.gpsimd.collective_compute(
                        "AllGather", ALU.bypass, replica_groups=RG,
                        ins=[ag_in.ap().opt()],
                        outs=[tables[l + 1].ap().opt()],
                    )

    nc.compile()
    return nc


def kernel(**inputs):
    x = _to_np(inputs["x"], np.float32)
    ei = _to_np(inputs["edge_index"], np.int64)
    batch = _to_np(inputs["batch"], np.int64)
    ew = _to_np(inputs["edge_weight"], np.float32)
    lin_W = _to_np(inputs["lin_W"], np.float32)
    lin_b = _to_np(inputs["lin_b"], np.float32)
    W1 = _to_np(inputs["W1"], np.float32)
    b1 = _to_np(inputs["b1"], np.float32)
    ln_gamma = _to_np(inputs["ln_gamma"], np.float32)
    ln_beta = _to_np(inputs["ln_beta"], np.float32)
    W2 = _to_np(inputs["W2"], np.float32)
    b2 = _to_np(inputs["b2"], np.float32)
    bn_gamma = _to_np(inputs["bn_gamma"], np.float32)
    bn_beta = _to_np(inputs["bn_beta"], np.float32)

    src, dst = ei[0], ei[1]
    plan, percore = _prep(x, src, dst, ew, batch)

    flags = dict(
        blin_zero=bool(np.all(lin_b == 0)),
        b1_zero=bool(np.all(b1 == 0)),
        b2_zero=bool(np.all(b2 == 0)),
        ln_triv=bool(np.all(ln_gamma == 1) and np.all(ln_beta == 0)),
        bng_one=bool(np.all(bn_gamma == 1)),
        bnb_zero=bool(np.all(bn_beta == 0)),
    )

    nc = _build(plan, flags)

    shared = dict(
        wlin=lin_W.astype(NP_BF16),
        blin=lin_b.reshape(1, 128).astype(NP_BF16),
        W1=W1.astype(NP_BF16),
        b1=b1.reshape(L, 1, 256).astype(NP_BF16),
        W2=W2.reshape(L, 2, 128, 128).transpose(0, 2, 1, 3).copy().astype(NP_BF16),
        b2=b2.reshape(L, 1, 128).astype(NP_BF16),
        lng=np.broadcast_to(ln_gamma.reshape(L, 1, 256),
                            (L, 128, 256)).copy().astype(np.float32),
        lnb=np.broadcast_to(ln_beta.reshape(L, 1, 256),
                            (L, 128, 256)).copy().astype(np.float32),
        bng=bn_gamma.reshape(L, 1, 128).astype(np.float32),
        bnb=bn_beta.reshape(L, 1, 128).astype(np.float32),
    )
    in_maps = []
    for c in range(C):
        m = dict(shared)
        m["x0T"] = percore["x0T"][c]
        m["gidx"] = percore["gidx"][c]
        m["ind"] = percore["ind"][c]
        m["poolind"] = percore["pool"][c]
        m["mask"] = percore["mask"][c]
        in_maps.append(m)

    res = run_bass_kernel_spmd(nc, in_maps, core_ids=list(range(C)))
    global _last_run
    _last_run = res

    xs = np.zeros((N, L * DIM), np.float32)
    pooled = np.zeros((G, L * DIM), np.float32)
    for c in range(C):
        r = res.results[c]
        xo = r["xs_out"]       # [L, 128, NGRP, 128]
        po = r["pool_out"]     # [L, 2, 128, 128]
        for l in range(L):
            sl = xo[l].transpose(1, 0, 2).reshape(NPAD, 128)[:NPC]
            xs[c * NPC:(c + 1) * NPC, l * DIM:(l + 1) * DIM] = sl
            pooled[:, l * DIM:(l + 1) * DIM] += po[l].reshape(256, 128)
    return pooled, xs


# revision 24
# speedup vs baseline: 1.2038x; 1.0243x over previous
"""GIN-style GNN encoder (3-layer message passing + FFN + norms + segment pool)
on 8 Trainium2 NeuronCores.

Strategy:
- Nodes are range-partitioned across the 8 cores (5000 each, padded to 5120).
  Edges are assigned to the core owning their *destination* node, so the
  scatter-add is core-local (no all-reduce over node features).
- Per core, local nodes are processed in 40 groups of 128; each group has two
  64-node windows. Edges are bucketed by (src-table-row < 32768) so gather
  indices fit int16, grouped by window, and padded to 128-edge chunks. The
  chunk counts per (bucket, window) are maxed over cores so all 8 cores run
  one identical SPMD program (pad chunks gather row 0 with all-zero
  indicators).
- Gather: dma_gather from a replicated bf16 node table in DRAM (the AllGather
  output of the previous layer). Scatter-add: per chunk, matmul
  (messages[e,f] as stationary)T @ indicator[e,64nodes] accumulating
  aggr^T in PSUM; the indicator carries the edge weight.
- FFN/LayerNorm run with nodes on partitions; BatchNorm statistics are
  one AllReduce of [1,256] per layer; pooling is an indicator matmul into
  per-core partials that the host sums.
"""

import os
import numpy as np
import ml_dtypes

import concourse.bass as bass
import concourse.bacc as bacc
import concourse.mybir as mybir
from concourse.bass_utils import run_bass_kernel_spmd
from concourse.tile import TileContext
from concourse.masks import make_identity
from concourse.tile_rust import add_dep_helper

# Problem constants (hardcoded per harness contract)
N = 40000
E = 640000
F_IN = 128
DIM = 128
L = 3
G = 256
EPS = 1e-5

C = 8            # cores
NPC = 5000       # real nodes per core
NGRP = 40        # groups of 128 (includes padding)
NPAD = NGRP * 128  # 5120
TBL = C * NPAD   # 40960 table rows
BSPLIT = 32768   # int16 bucket split in table-row space
W = 64           # indicator window (nodes)
WPG = 2          # windows per group
NWIN = NGRP * WPG  # 80 windows/core
SC = 8   # chunks per gather super-call (1024 idxs; >=2048 overflows the SWDGE ring and hangs)

BF16 = mybir.dt.bfloat16
F32 = mybir.dt.float32
I16 = mybir.dt.int16

AF = mybir.ActivationFunctionType
ALU = mybir.AluOpType

NP_BF16 = ml_dtypes.bfloat16

_last_run = None  # BassKernelResults of the most recent kernel() call


def _to_np(a, dtype=None):
    a = np.asarray(a)
    if dtype is not None:
        a = a.astype(dtype)
    return a


def _prep(x, src, dst, ew, batch):
    """Host-side sharding. Returns (plan, per-core arrays)."""
    core = dst // NPC
    ldst = dst - core * NPC
    win = ldst // W
    wcol = ldst - win * W
    trow = (src // NPC) * NPAD + (src % NPC)
    bucket = (trow >= BSPLIT).astype(np.int64)

    gid = (core * 2 + bucket) * NWIN + win
    cnt = np.bincount(gid, minlength=C * 2 * NWIN).reshape(C, 2, NWIN)
    cpw = np.maximum(np.ceil(cnt / 128.0).astype(np.int64).max(axis=0), 1)  # [2, NWIN]
    KA = int(cpw[0].sum())
    KB = int(cpw[1].sum())
    K = KA + KB
    chunk_base = np.zeros((2, NWIN), np.int64)
    chunk_base[0, :] = np.concatenate([[0], np.cumsum(cpw[0])[:-1]])
    chunk_base[1, :] = KA + np.concatenate([[0], np.cumsum(cpw[1])[:-1]])

    # per-edge position within its (core,bucket,win) run
    order = np.argsort(gid, kind="stable")
    sg = gid[order]
    runstart = np.zeros(len(sg), np.int64)
    newrun = np.ones(len(sg), bool)
    newrun[1:] = sg[1:] != sg[:-1]
    runidx = np.flatnonzero(newrun)
    runstart[runidx] = runidx
    runstart = np.maximum.accumulate(runstart)
    pos_sorted = np.arange(len(sg)) - runstart
    pos = np.empty(len(sg), np.int64)
    pos[order] = pos_sorted

    chunk = chunk_base[bucket, win] + pos // 128
    prow = pos % 128

    gidx_all = []
    ind_all = []
    for c in range(C):
        m = core == c
        lin = np.zeros(K * 128, np.int32)
        lin[chunk[m] * 128 + prow[m]] = trow[m] - bucket[m] * BSPLIT
        assert lin.max() < 32768 and lin.min() >= 0
        # wrap for dma_gather: index i -> partition i%16, col i//16; replicate x8
        arr16 = lin.astype(np.int16).reshape(K * 8, 16).T  # [16, K*8]
        gidx_all.append(np.tile(arr16, (8, 1)))            # [128, K*8]
        ind = np.zeros((128, K, W), NP_BF16)
        ind[prow[m], chunk[m], wcol[m]] = ew[m].astype(NP_BF16)
        ind_all.append(ind)

    # pooling indicators + node mask
    pool_all = []
    mask_all = []
    for c in range(C):
        pi = np.zeros((128, NGRP, 2, 128), NP_BF16)
        b_loc = batch[c * NPC:(c + 1) * NPC]
        nl = np.arange(NPC)
        t = nl // 128
        p = nl % 128
        pi[p, t, b_loc // 128, b_loc % 128] = 1.0
        pool_all.append(pi)
        mk = np.zeros((128, NGRP), NP_BF16)
        mk[p, t] = 1.0
        mask_all.append(mk)

    # x transposed slices, bf16
    x0T_all = []
    for c in range(C):
        s = np.zeros((128, NPAD), NP_BF16)
        xs = x[c * NPC:(c + 1) * NPC]  # [5000, F_IN]
        # node local n = t*128 + p -> column t*128+p? layout [p, t, f] flattened as
        # [128, NGRP*128] with column t*128+f. We store as [f? no]:
        # lhsT for proj needs [f, node] per tile: columns = node within tile.
        # Use layout [128 f, NPAD nodes]: s[f, n] = x[n, f]
        s[:, :NPC] = xs.T.astype(NP_BF16)
        x0T_all.append(s)

    plan = dict(cpw=cpw, KA=KA, KB=KB, K=K, chunk_base=chunk_base)
    percore = dict(gidx=gidx_all, ind=ind_all, pool=pool_all, mask=mask_all,
                   x0T=x0T_all)
    return plan, percore


def _build(plan, flags, dbg=None):
    """Build the SPMD Bass program (identical across cores)."""
    cpw = plan["cpw"]
    KA, KB, K = plan["KA"], plan["KB"], plan["K"]
    if dbg is None:
        dbg = {}
    NL = dbg.get("nl", L)          # layers to actually run
    DO_FFN = dbg.get("ffn", True)  # run FFN block per group
    DO_TAIL = dbg.get("tail", True)  # stats AR + BN apply + pool + AG
    NO_MM = dbg.get("nomm", False)   # skip seg matmuls/evictions
    NO_IND = dbg.get("noind", False)  # skip indicator DMAs
    NO_CONST = dbg.get("noconst", False)  # skip ident/poolind/mask/eps/ones loads
    NO_PROJ = dbg.get("noproj", False)  # skip proj matmuls (raw x -> table)
    NO_LONGPS = dbg.get("nolongps", False)  # skip stats/pool psum allocation
    DVE_MM = dbg.get("dvemm", False)  # replace seg MMs with DVE consumes
    DUMP = dbg.get("dump", False)    # dump stage intermediates (layer 0)

    nc = bacc.Bacc("TRN2", target_bir_lowering=False, num_swdge_queues=4)

    # ---- DRAM parameters ----
    p_x0T = nc.declare_dram_parameter("x0T", [128, NPAD], BF16, isOutput=False)
    p_gidx = nc.declare_dram_parameter("gidx", [128, K * 8], I16, isOutput=False)
    p_ind = nc.declare_dram_parameter("ind", [128, K, W], BF16, isOutput=False)
    p_pool = nc.declare_dram_parameter("poolind", [128, NGRP, 2, 128], BF16,
                                       isOutput=False)
    p_mask = nc.declare_dram_parameter("mask", [128, NGRP], BF16, isOutput=False)
    p_wlin = nc.declare_dram_parameter("wlin", [128, 128], BF16, isOutput=False)
    p_blin = nc.declare_dram_parameter("blin", [1, 128], BF16, isOutput=False)
    p_W1 = nc.declare_dram_parameter("W1", [L, 128, 256], BF16, isOutput=False)
    p_b1 = nc.declare_dram_parameter("b1", [L, 1, 256], BF16, isOutput=False)
    # W2 stored as [L, 128, 2, 128]: [:, p, k, :] = W2[l, k*128+p, :]
    p_W2 = nc.declare_dram_parameter("W2", [L, 128, 2, 128], BF16, isOutput=False)
    p_b2 = nc.declare_dram_parameter("b2", [L, 1, 128], BF16, isOutput=False)
    p_lng = nc.declare_dram_parameter("lng", [L, 128, 256], F32, isOutput=False)
    p_lnb = nc.declare_dram_parameter("lnb", [L, 128, 256], F32, isOutput=False)
    p_bng = nc.declare_dram_parameter("bng", [L, 1, 128], F32, isOutput=False)
    p_bnb = nc.declare_dram_parameter("bnb", [L, 1, 128], F32, isOutput=False)
    o_xs = nc.declare_dram_parameter("xs_out", [L, 128, NGRP, 128], F32,
                                     isOutput=True)
    o_pool = nc.declare_dram_parameter("pool_out", [L, 2, 128, 128], F32,
                                       isOutput=True)
    if dbg and dbg.get("dump", False):
        o_dtbl = nc.declare_dram_parameter("dbg_tbl", [TBL, 128], BF16,
                                           isOutput=True)
        o_daggr = nc.declare_dram_parameter("dbg_aggrA", [128, NWIN * W], F32,
                                            isOutput=True)
        o_dysb = nc.declare_dram_parameter("dbg_ysb", [128, NGRP, 128], F32,
                                           isOutput=True)

    # internal DRAM
    ag_in = nc.dram_tensor("ag_in", [NPAD, 128], BF16)
    tables = [nc.dram_tensor(f"table{l}", [TBL, 128], BF16, addr_space="Shared")
              for l in range(L)]
    ar_ins = [nc.dram_tensor(f"ar_in{l}", [1, 256], F32) for l in range(L)]
    ar_outs = [nc.dram_tensor(f"ar_out{l}", [1, 256], F32, addr_space="Shared")
               for l in range(L)]

    RG = [list(range(C))]

    # chunk metadata: list over k of (bucket, window)
    chunk_win = np.zeros(K, np.int64)
    for b in range(2):
        for w in range(NWIN):
            base = plan["chunk_base"][b, w]
            chunk_win[base:base + cpw[b, w]] = w
    # supercalls within each bucket
    def supercalls(lo, hi):
        out = []
        k = lo
        while k < hi:
            out.append((k, min(SC, hi - k)))
            k += SC
        return out

    calls_A = supercalls(0, KA)
    calls_B = supercalls(KA, K)

    with TileContext(nc) as tc:
        with (
            tc.tile_pool(name="const", bufs=1) as cpool,
            tc.tile_pool(name="params", bufs=2) as ppool,
            tc.tile_pool(name="indp", bufs=2) as indp,
            tc.tile_pool(name="goutp", bufs=2) as goutp,
            tc.tile_pool(name="segps", bufs=2, space="PSUM") as segps,
            tc.tile_pool(name="ffnps", bufs=2, space="PSUM") as ffnps,
            tc.tile_pool(name="longps", bufs=1, space="PSUM") as longps,
            tc.tile_pool(name="work", bufs=3) as work,
            tc.tile_pool(name="bigbuf", bufs=1) as bigp,
        ):
            # ---- constants ----
            ident = cpool.tile([128, 128], BF16)
            ones_bf = cpool.tile([1, 128], BF16)
            ones_f = cpool.tile([1, 128], F32)
            epsc = cpool.tile([128, 1], F32)
            poolind = cpool.tile([128, NGRP, 2, 128], BF16)
            maskt = cpool.tile([128, NGRP], BF16)
            if not NO_CONST:
                make_identity(nc, ident[:, :])
                nc.vector.memset(ones_bf[:, :], 1.0)
                nc.vector.memset(ones_f[:, :], 1.0)
                nc.vector.memset(epsc[:, :], EPS)
                nc.sync.dma_start(out=poolind[:, :, :, :], in_=p_pool[:, :, :, :])
                nc.sync.dma_start(out=maskt[:, :], in_=p_mask[:, :])
            gidx = cpool.tile([128, K * 8], I16)
            nc.sync.dma_start(out=gidx[:, :], in_=p_gidx[:, :])
            wlin = cpool.tile([128, 128], BF16)
            nc.sync.dma_start(out=wlin[:, :], in_=p_wlin[:, :])
            blin = cpool.tile([1, 128], BF16)
            nc.sync.dma_start(out=blin[:, :], in_=p_blin[:, :])

            # ---- projection: x1 = x0 @ lin_W + lin_b -> bf16 table0 ----
            x0T = bigp.tile([128, NPAD], BF16, tag="x0T")
            nc.sync.dma_start(out=x0T[:, :], in_=p_x0T[:, :])
            xnew_bf = bigp.tile([128, NGRP, 128], BF16, tag="xnewbf")
            for t in range(0 if NO_PROJ else NGRP):
                ps = ffnps.tile([128, 128], F32, tag="hps")
                nc.tensor.matmul(ps[:, :], lhsT=x0T[:, t * 128:(t + 1) * 128],
                                 rhs=wlin[:, :], start=True,
                                 stop=flags["blin_zero"])
                if not flags["blin_zero"]:
                    nc.tensor.matmul(ps[:, :], lhsT=ones_bf[:, :],
                                     rhs=blin[:, :], start=False, stop=True)
                nc.scalar.copy(out=xnew_bf[:, t, :], in_=ps[:, :])
            if NO_PROJ:
                nc.sync.dma_start(
                    out=ag_in[:, :].rearrange("(t p) f -> p (t f)", p=128),
                    in_=x0T[:, :],
                )
            else:
                nc.sync.dma_start(
                    out=ag_in[:, :].rearrange("(t p) f -> p t f", p=128),
                    in_=xnew_bf[:, :, :],
                )
            ag0 = nc.gpsimd.collective_compute(
                "AllGather", ALU.bypass, replica_groups=RG,
                ins=[ag_in.ap().opt()], outs=[tables[0].ap().opt()],
            )
            ag_insts = [ag0, None, None]
            if DUMP:
                tcp = nc.sync.dma_start(out=o_dtbl[:, :], in_=tables[0][:, :])
                add_dep_helper(tcp.ins, ag0.ins, sync=True, reason="dbg tbl dump")

            # ---- layers ----
            aggrA = bigp.tile([128, NWIN * W], F32, tag="aggrA")
            y_sb = bigp.tile([128, NGRP, 128], F32, tag="ysb")
            xnew_f = bigp.tile([128, NGRP, 128], F32, tag="xnewf")

            for l in range(NL):
                # Hard inter-layer barrier: the big SBUF buffers (xnew_f,
                # y_sb, aggrA, xnew_bf) are reused across layers and Tile
                # misses some cross-layer WAR edges on HW (sim passes but
                # HW shows element-level races). ~2us each, negligible.
                tc.strict_bb_all_engine_barrier()
                table = tables[l]
                tblA = table[0:BSPLIT, :]
                tblB = table[BSPLIT:TBL, :]

                w1 = ppool.tile([128, 256], BF16, tag="w1")
                nc.sync.dma_start(out=w1[:, :], in_=p_W1[l, :, :])
                w2 = ppool.tile([128, 2, 128], BF16, tag="w2")
                nc.sync.dma_start(out=w2[:, :, :], in_=p_W2[l, :, :, :])
                b1r = ppool.tile([1, 256], BF16, tag="b1r")
                nc.sync.dma_start(out=b1r[:, :], in_=p_b1[l, :, :])
                b2r = ppool.tile([1, 128], BF16, tag="b2r")
                nc.sync.dma_start(out=b2r[:, :], in_=p_b2[l, :, :])
                if not flags["ln_triv"]:
                    lng = ppool.tile([128, 256], F32, tag="lng")
                    nc.sync.dma_start(out=lng[:, :], in_=p_lng[l, :, :])
                    lnb = ppool.tile([128, 256], F32, tag="lnb")
                    nc.sync.dma_start(out=lnb[:, :], in_=p_lnb[l, :, :])
                bng = ppool.tile([1, 128], F32, tag="bng")
                nc.sync.dma_start(out=bng[:, :], in_=p_bng[l, :, :])
                bnb = ppool.tile([1, 128], F32, tag="bnb")
                nc.sync.dma_start(out=bnb[:, :], in_=p_bnb[l, :, :])

                if not NO_LONGPS:
                    stats_ps = longps.tile([1, 256], F32, tag="stats")
                    pool_ps = longps.tile([128, 256], F32, tag="poolps")

                # ---------- seg-matmul passes ----------
                def run_pass(calls, tbl_ap, pass_b, dep_inst):
                    cur_ps = {}

                    def win_ps(w):
                        g = w // WPG
                        if g not in cur_ps:
                            cur_ps[g] = segps.tile([128, 128], F32, tag="seg", name=f"seg{g}")
                        return cur_ps[g], (w % WPG) * W

                    done_in_win = {}
                    for (k0, nch) in calls:
                        gout = goutp.tile([128, SC, 128], BF16, tag="gout")
                        indt = indp.tile([128, SC, W], BF16, tag="ind")
                        if not NO_IND:
                            nc.sync.dma_start(out=indt[:, 0:nch, :],
                                              in_=p_ind[:, k0:k0 + nch, :])
                        g_inst = nc.gpsimd.dma_gather(
                            out_ap=gout[:, 0:nch, :],
                            in_ap=tbl_ap,
                            idxs_ap=gidx[:, k0 * 8:(k0 + nch) * 8],
                            num_idxs=nch * 128,
                            num_idxs_reg=nch * 128,
                            elem_size=128,
                            queue_num=(k0 // SC) % 4,
                            single_packet=False,
                        )
                        if dep_inst is not None:
                            add_dep_helper(g_inst.ins, dep_inst.ins, sync=True,
                                           reason="gather reads AG table")
                        if NO_MM:
                            continue
                        for j in range(nch):
                            k = k0 + j
                            w = int(chunk_win[k])
                            b = 1 if pass_b else 0
                            first = done_in_win.get(w, 0) == 0
                            done_in_win[w] = done_in_win.get(w, 0) + 1
                            last = done_in_win[w] == cpw[b, w]
                            if DVE_MM:
                                sc1 = work.tile([128, 64], F32, tag="dvemm",
                                                name=f"dv{k}{pass_b}")
                                nc.vector.tensor_tensor(
                                    out=sc1[:, :], in0=gout[:, j, 0:64],
                                    in1=indt[:, j, :], op=ALU.add)
                                if last and w % WPG == 1:
                                    yield w // WPG, None
                                continue
                            ps, coff = win_ps(w)
                            # PSUM start=True zeroes the whole 2KB bank, so
                            # only the first matmul touching the tile starts.
                            nc.tensor.matmul(
                                ps[:, coff:coff + W],
                                lhsT=gout[:, j, :],
                                rhs=indt[:, j, :],
                                start=(first and w % WPG == 0),
                                stop=(last and w % WPG == 1),
                                skip_group_check=True,
                            )
                            if last and w % WPG == 1:
                                g = w // WPG
                                yield g, cur_ps.pop(g)

                # pass A: evict into aggrA
                for g, ps in run_pass(calls_A, tblA, False, ag_insts[l]):
                    if ps is None:
                        continue
                    nc.scalar.copy(out=aggrA[:, g * 128:(g + 1) * 128],
                                   in_=ps[:, :])

                if DUMP and l == 0:
                    nc.sync.dma_start(out=o_daggr[:, :], in_=aggrA[:, :])
                # pass B: combine + FFN per group
                for g, psB in run_pass(calls_B, tblB, True, ag_insts[l]):
                    aggrT_f = work.tile([128, 128], F32, tag="aggrTf")
                    if psB is None:
                        nc.vector.memset(aggrT_f[:, :], 0.0)
                    else:
                        nc.vector.tensor_tensor(
                            out=aggrT_f[:, :], in0=psB[:, :],
                            in1=aggrA[:, g * 128:(g + 1) * 128], op=ALU.add)
                    if not DO_FFN:
                        nc.scalar.copy(out=y_sb[:, g, :], in_=aggrT_f[:, :])
                        continue
                    aggrT_bf = work.tile([128, 128], BF16, tag="aggrTbf")
                    nc.scalar.copy(out=aggrT_bf[:, :], in_=aggrT_f[:, :])

                    h_ps = ffnps.tile([128, 256], F32, tag="hps")
                    nc.tensor.matmul(h_ps[:, :], lhsT=aggrT_bf[:, :],
                                     rhs=w1[:, :], start=True,
                                     stop=flags["b1_zero"])
                    if not flags["b1_zero"]:
                        nc.tensor.matmul(h_ps[:, :], lhsT=ones_bf[:, :],
                                         rhs=b1r[:, :], start=False, stop=True)
                    st6 = work.tile([128, 6], F32, tag="st6")
                    nc.vector.bn_stats(st6[:, :], h_ps[:, :])
                    mv = work.tile([128, 2], F32, tag="mv")
                    nc.vector.bn_aggr(mv[:, :], st6[:, :])
                    sd = work.tile([128, 1], F32, tag="sd")
                    nc.scalar.activation(sd[:, :], mv[:, 1:2], AF.Sqrt, bias=epsc[:, :])
                    inv = work.tile([128, 1], F32, tag="inv")
                    nc.vector.reciprocal(inv[:, :], sd[:, :])
                    t_sb = work.tile([128, 256], F32, tag="tsb")
                    nc.vector.tensor_scalar(
                        out=t_sb[:, :], in0=h_ps[:, :],
                        scalar1=mv[:, 0:1], scalar2=inv[:, :],
                        op0=ALU.subtract, op1=ALU.mult)
                    if not flags["ln_triv"]:
                        u = work.tile([128, 256], F32, tag="lnu")
                        nc.vector.tensor_tensor(out=u[:, :], in0=t_sb[:, :],
                                                in1=lng[:, :], op=ALU.mult)
                        nc.vector.tensor_tensor(out=u[:, :], in0=u[:, :],
                                                in1=lnb[:, :], op=ALU.add)
                        relu_src = u
                    else:
                        relu_src = t_sb
                    rh_bf = work.tile([128, 256], BF16, tag="rhbf")
                    nc.scalar.activation(rh_bf[:, :], relu_src[:, :], AF.Relu)

                    tr0 = ffnps.tile([128, 128], BF16, tag="fsm")
                    nc.tensor.transpose(tr0[:, :], rh_bf[:, 0:128], ident[:, :])
                    tr1 = ffnps.tile([128, 128], BF16, tag="fsm")
                    nc.tensor.transpose(tr1[:, :], rh_bf[:, 128:256], ident[:, :])
                    rhT0 = work.tile([128, 128], BF16, tag="rhT0")
                    nc.vector.tensor_copy(out=rhT0[:, :], in_=tr0[:, :])
                    rhT1 = work.tile([128, 128], BF16, tag="rhT1")
                    nc.scalar.copy(out=rhT1[:, :], in_=tr1[:, :])

                    y_ps = ffnps.tile([128, 128], F32, tag="fsm")
                    nc.tensor.matmul(y_ps[:, :], lhsT=rhT0[:, :],
                                     rhs=w2[:, 0, :], start=True, stop=False)
                    nc.tensor.matmul(y_ps[:, :], lhsT=rhT1[:, :],
                                     rhs=w2[:, 1, :], start=False, stop=False)
                    if not flags["b2_zero"]:
                        nc.tensor.matmul(y_ps[:, :], lhsT=ones_bf[:, :],
                                         rhs=b2r[:, :], start=False, stop=False)
                    nc.tensor.matmul(y_ps[:, :], lhsT=aggrT_bf[:, :],
                                     rhs=ident[:, :], start=False, stop=True)

                    nc.scalar.activation(y_sb[:, g, :], y_ps[:, :], AF.Relu)
                    ybf = work.tile([128, 128], BF16, tag="ybf")
                    nc.scalar.copy(out=ybf[:, :], in_=y_sb[:, g, :])
                    ysq = work.tile([128, 128], BF16, tag="ysq")
                    nc.scalar.activation(ysq[:, :], y_sb[:, g, :], AF.Square)
                    nc.tensor.matmul(stats_ps[:, 0:128],
                                     lhsT=maskt[:, g:g + 1], rhs=ybf[:, :],
                                     start=(g == 0), stop=False,
                                     skip_group_check=True)
                    nc.tensor.matmul(stats_ps[:, 128:256],
                                     lhsT=maskt[:, g:g + 1], rhs=ysq[:, :],
                                     start=False, stop=(g == NGRP - 1),
                                     skip_group_check=True)

                if DUMP and l == 0:
                    nc.sync.dma_start(out=o_dysb[:, :, :], in_=y_sb[:, :, :])
                # ---------- BN stats all-reduce ----------
                if not DO_TAIL:
                    nc.sync.dma_start(out=o_xs[l, :, :, :], in_=y_sb[:, :, :])
                    continue
                st_sb = work.tile([1, 256], F32, tag="stsb")
                nc.scalar.copy(out=st_sb[:, :], in_=stats_ps[:, :])
                nc.gpsimd.dma_start(out=ar_ins[l][:, :], in_=st_sb[:, :])
                ar_inst = nc.gpsimd.collective_compute(
                    "AllReduce", ALU.add, replica_groups=RG,
                    ins=[ar_ins[l].ap().opt()], outs=[ar_outs[l].ap().opt()],
                )
                stg = work.tile([1, 256], F32, tag="stg")
                stg_rd = nc.sync.dma_start(out=stg[:, :], in_=ar_outs[l][:, :])
                add_dep_helper(stg_rd.ins, ar_inst.ins, sync=True,
                               reason="stats read after AllReduce")
                st2 = work.tile([1, 256], F32, tag="st2")  # [s | t]
                mrow = work.tile([1, 128], F32, tag="mrow")
                nc.vector.tensor_scalar_mul(mrow[:, :], stg[:, 0:128], 1.0 / N)
                qrow = work.tile([1, 128], F32, tag="qrow")
                nc.vector.tensor_scalar_mul(qrow[:, :], stg[:, 128:256], 1.0 / N)
                msq = work.tile([1, 128], F32, tag="msq")
                nc.vector.tensor_tensor(out=msq[:, :], in0=mrow[:, :],
                                        in1=mrow[:, :], op=ALU.mult)
                vrow = work.tile([1, 128], F32, tag="vrow")
                nc.vector.tensor_tensor(out=vrow[:, :], in0=qrow[:, :],
                                        in1=msq[:, :], op=ALU.subtract)
                sdr = work.tile([1, 128], F32, tag="sdr")
                nc.scalar.activation(sdr[:, :], vrow[:, :], AF.Sqrt, bias=epsc[0:1, :])
                invr = work.tile([1, 128], F32, tag="invr")
                nc.vector.reciprocal(invr[:, :], sdr[:, :])
                if flags["bng_one"]:
                    nc.vector.tensor_copy(out=st2[:, 0:128], in_=invr[:, :])
                else:
                    nc.vector.tensor_tensor(out=st2[:, 0:128], in0=invr[:, :],
                                            in1=bng[:, :], op=ALU.mult)
                ms = work.tile([1, 128], F32, tag="ms")
                nc.vector.tensor_tensor(out=ms[:, :], in0=mrow[:, :],
                                        in1=st2[:, 0:128], op=ALU.mult)
                if flags["bnb_zero"]:
                    nc.vector.tensor_scalar_mul(st2[:, 128:256], ms[:, :], -1.0)
                else:
                    nc.vector.tensor_tensor(out=st2[:, 128:256], in0=bnb[:, :],
                                            in1=ms[:, :], op=ALU.subtract)
                bc_ps = ffnps.tile([128, 256], F32, tag="hps")
                nc.tensor.matmul(bc_ps[:, :], lhsT=ones_f[:, :], rhs=st2[:, :],
                                 start=True, stop=True)
                st_bc = work.tile([128, 256], F32, tag="stbc")
                nc.scalar.copy(out=st_bc[:, :], in_=bc_ps[:, :])

                # ---------- BN apply + pool ----------
                for g in range(NGRP):
                    tmp = work.tile([128, 128], F32, tag="bntmp")
                    nc.vector.tensor_tensor(out=tmp[:, :], in0=y_sb[:, g, :],
                                            in1=st_bc[:, 0:128], op=ALU.mult)
                    nc.vector.tensor_tensor(out=xnew_f[:, g, :], in0=tmp[:, :],
                                            in1=st_bc[:, 128:256], op=ALU.add)
                    nc.scalar.copy(out=xnew_bf[:, g, :], in_=xnew_f[:, g, :])
                    nc.tensor.matmul(pool_ps[:, 0:128], lhsT=poolind[:, g, 0, :],
                                     rhs=xnew_bf[:, g, :], start=(g == 0),
                                     stop=False, skip_group_check=True)
                    nc.tensor.matmul(pool_ps[:, 128:256], lhsT=poolind[:, g, 1, :],
                                     rhs=xnew_bf[:, g, :], start=False,
                                     stop=(g == NGRP - 1), skip_group_check=True)

                nc.sync.dma_start(out=o_xs[l, :, :, :], in_=xnew_f[:, :, :])
                pl0 = work.tile([128, 128], F32, tag="pl0")
                nc.scalar.copy(out=pl0[:, :], in_=pool_ps[:, 0:128])
                nc.sync.dma_start(out=o_pool[l, 0, :, :], in_=pl0[:, :])
                pl1 = work.tile([128, 128], F32, tag="pl1")
                nc.scalar.copy(out=pl1[:, :], in_=pool_ps[:, 128:256])
                nc.sync.dma_start(out=o_pool[l, 1, :, :], in_=pl1[:, :])

                if l < L - 1:
                    nc.sync.dma_start(
                        out=ag_in[:, :].rearrange("(t p) f -> p t f", p=128),
                        in_=xnew_bf[:, :, :],
                    )
                    ag_insts[l + 1] = nc.gpsimd.collective_compute(
                        "AllGather", ALU.bypass, replica_groups=RG,
                        ins=[ag_in.ap().opt()],
                        outs=[tables[l + 1].ap().opt()],
                    )

    nc.compile()
    return nc


def kernel(**inputs):
    x = _to_np(inputs["x"], np.float32)
    ei = _to_np(inputs["edge_index"], np.int64)
    batch = _to_np(inputs["batch"], np.int64)
    ew = _to_np(inputs["edge_weight"], np.float32)
    lin_W = _to_np(inputs["lin_W"], np.float32)
    lin_b = _to_np(inputs["lin_b"], np.float32)
    W1 = _to_np(inputs["W1"], np.float32)
    b1 = _to_np(inputs["b1"], np.float32)
    ln_gamma = _to_np(inputs["ln_gamma"], np.float32)
    ln_beta = _to_np(inputs["ln_beta"], np.float32)
    W2 = _to_np(inputs["W2"], np.float32)
    b2 = _to_np(inputs["b2"], np.float32)
    bn_gamma = _to_np(inputs["bn_gamma"], np.float32)
    bn_beta = _to_np(inputs["bn_beta"], np.float32)

    src, dst = ei[0], ei[1]
    plan, percore = _prep(x, src, dst, ew, batch)

    flags = dict(
        blin_zero=bool(np.all(lin_b == 0)),
        b1_zero=bool(np.all(b1 == 0)),
        b2_zero=bool(np.all(b2 == 0)),
        ln_triv=bool(np.all(ln_gamma == 1) and np.all(ln_beta == 0)),
        bng_one=bool(np.all(bn_gamma == 1)),
        bnb_zero=bool(np.all(bn_beta == 0)),
    )

    nc = _build(plan, flags)

    shared = dict(
        wlin=lin_W.astype(NP_BF16),
        blin=lin_b.reshape(1, 128).astype(NP_BF16),
        W1=W1.astype(NP_BF16),
        b1=b1.reshape(L, 1, 256).astype(NP_BF16),
        W2=W2.reshape(L, 2, 128, 128).transpose(0, 2, 1, 3).copy().astype(NP_BF16),
        b2=b2.reshape(L, 1, 128).astype(NP_BF16),
        lng=np.broadcast_to(ln_gamma.reshape(L, 1, 256),
                            (L, 128, 256)).copy().astype(np.float32),
        lnb=np.broadcast_to(ln_beta.reshape(L, 1, 256),
                            (L, 128, 256)).copy().astype(np.float32),
        bng=bn_gamma.reshape(L, 1, 128).astype(np.float32),
        bnb=bn_beta.reshape(L, 1, 128).astype(np.float32),
    )
    in_maps = []
    for c in range(C):
        m = dict(shared)
        m["x0T"] = percore["x0T"][c]
        m["gidx"] = percore["gidx"][c]
        m["ind"] = percore["ind"][c]
        m["poolind"] = percore["pool"][c]
        m["mask"] = percore["mask"][c]
        in_maps.append(m)

    res = run_bass_kernel_spmd(nc, in_maps, core_ids=list(range(C)))
    global _last_run
    _last_run = res

    xs = np.zeros((N, L * DIM), np.float32)
    pooled = np.zeros((G, L * DIM), np.float32)
    for c in range(C):
        r = res.results[c]
        xo = r["xs_out"]       # [L, 128, NGRP, 128]
        po = r["pool_out"]     # [L, 2, 128, 128]
        for l in range(L):
            sl = xo[l].transpose(1, 0, 2).reshape(NPAD, 128)[:NPC]
            xs[c * NPC:(c + 1) * NPC, l * DIM:(l + 1) * DIM] = sl
            pooled[:, l * DIM:(l + 1) * DIM] += po[l].reshape(256, 128)
    return pooled, xs


# revision 25
# speedup vs baseline: 1.2096x; 1.0048x over previous
"""GIN-style GNN encoder (3-layer message passing + FFN + norms + segment pool)
on 8 Trainium2 NeuronCores.

Strategy:
- Nodes are range-partitioned across the 8 cores (5000 each, padded to 5120).
  Edges are assigned to the core owning their *destination* node, so the
  scatter-add is core-local (no all-reduce over node features).
- Per core, local nodes are processed in 40 groups of 128; each group has two
  64-node windows. Edges are bucketed by (src-table-row < 32768) so gather
  indices fit int16, grouped by window, and padded to 128-edge chunks. The
  chunk counts per (bucket, window) are maxed over cores so all 8 cores run
  one identical SPMD program (pad chunks gather row 0 with all-zero
  indicators).
- Gather: dma_gather from a replicated bf16 node table in DRAM (the AllGather
  output of the previous layer). Scatter-add: per chunk, matmul
  (messages[e,f] as stationary)T @ indicator[e,64nodes] accumulating
  aggr^T in PSUM; the indicator carries the edge weight.
- FFN/LayerNorm run with nodes on partitions; BatchNorm statistics are
  one AllReduce of [1,256] per layer; pooling is an indicator matmul into
  per-core partials that the host sums.
"""

import os
import numpy as np
import ml_dtypes

import concourse.bass as bass
import concourse.bacc as bacc
import concourse.mybir as mybir
from concourse.bass_utils import run_bass_kernel_spmd
from concourse.tile import TileContext
from concourse.masks import make_identity
from concourse.tile_rust import add_dep_helper

# Problem constants (hardcoded per harness contract)
N = 40000
E = 640000
F_IN = 128
DIM = 128
L = 3
G = 256
EPS = 1e-5

C = 8            # cores
NPC = 5000       # real nodes per core
NGRP = 40        # groups of 128 (includes padding)
NPAD = NGRP * 128  # 5120
TBL = C * NPAD   # 40960 table rows
BSPLIT = 32768   # int16 bucket split in table-row space
W = 64           # indicator window (nodes)
WPG = 2          # windows per group
NWIN = NGRP * WPG  # 80 windows/core
SC = 8   # chunks per gather super-call (1024 idxs; >=2048 overflows the SWDGE ring and hangs)

BF16 = mybir.dt.bfloat16
F32 = mybir.dt.float32
I16 = mybir.dt.int16

AF = mybir.ActivationFunctionType
ALU = mybir.AluOpType

NP_BF16 = ml_dtypes.bfloat16

_last_run = None  # BassKernelResults of the most recent kernel() call


def _to_np(a, dtype=None):
    a = np.asarray(a)
    if dtype is not None:
        a = a.astype(dtype)
    return a


def _prep(x, src, dst, ew, batch):
    """Host-side sharding. Returns (plan, per-core arrays)."""
    core = dst // NPC
    ldst = dst - core * NPC
    win = ldst // W
    wcol = ldst - win * W
    trow = (src // NPC) * NPAD + (src % NPC)
    bucket = (trow >= BSPLIT).astype(np.int64)

    gid = (core * 2 + bucket) * NWIN + win
    cnt = np.bincount(gid, minlength=C * 2 * NWIN).reshape(C, 2, NWIN)
    cpw = np.maximum(np.ceil(cnt / 128.0).astype(np.int64).max(axis=0), 1)  # [2, NWIN]
    KA = int(cpw[0].sum())
    KB = int(cpw[1].sum())
    K = KA + KB
    chunk_base = np.zeros((2, NWIN), np.int64)
    chunk_base[0, :] = np.concatenate([[0], np.cumsum(cpw[0])[:-1]])
    chunk_base[1, :] = KA + np.concatenate([[0], np.cumsum(cpw[1])[:-1]])

    # per-edge position within its (core,bucket,win) run
    order = np.argsort(gid, kind="stable")
    sg = gid[order]
    runstart = np.zeros(len(sg), np.int64)
    newrun = np.ones(len(sg), bool)
    newrun[1:] = sg[1:] != sg[:-1]
    runidx = np.flatnonzero(newrun)
    runstart[runidx] = runidx
    runstart = np.maximum.accumulate(runstart)
    pos_sorted = np.arange(len(sg)) - runstart
    pos = np.empty(len(sg), np.int64)
    pos[order] = pos_sorted

    chunk = chunk_base[bucket, win] + pos // 128
    prow = pos % 128

    gidx_all = []
    ind_all = []
    for c in range(C):
        m = core == c
        lin = np.zeros(K * 128, np.int32)
        lin[chunk[m] * 128 + prow[m]] = trow[m] - bucket[m] * BSPLIT
        assert lin.max() < 32768 and lin.min() >= 0
        # wrap for dma_gather: index i -> partition i%16, col i//16; replicate x8
        arr16 = lin.astype(np.int16).reshape(K * 8, 16).T  # [16, K*8]
        gidx_all.append(np.tile(arr16, (8, 1)))            # [128, K*8]
        ind = np.zeros((128, K, W), NP_BF16)
        ind[prow[m], chunk[m], wcol[m]] = ew[m].astype(NP_BF16)
        ind_all.append(ind)

    # pooling indicators + node mask
    pool_all = []
    mask_all = []
    for c in range(C):
        pi = np.zeros((128, NGRP, 2, 128), NP_BF16)
        b_loc = batch[c * NPC:(c + 1) * NPC]
        nl = np.arange(NPC)
        t = nl // 128
        p = nl % 128
        pi[p, t, b_loc // 128, b_loc % 128] = 1.0
        pool_all.append(pi)
        mk = np.zeros((128, NGRP), NP_BF16)
        mk[p, t] = 1.0
        mask_all.append(mk)

    # x transposed slices, bf16
    x0T_all = []
    for c in range(C):
        s = np.zeros((128, NPAD), NP_BF16)
        xs = x[c * NPC:(c + 1) * NPC]  # [5000, F_IN]
        # node local n = t*128 + p -> column t*128+p? layout [p, t, f] flattened as
        # [128, NGRP*128] with column t*128+f. We store as [f? no]:
        # lhsT for proj needs [f, node] per tile: columns = node within tile.
        # Use layout [128 f, NPAD nodes]: s[f, n] = x[n, f]
        s[:, :NPC] = xs.T.astype(NP_BF16)
        x0T_all.append(s)

    plan = dict(cpw=cpw, KA=KA, KB=KB, K=K, chunk_base=chunk_base)
    percore = dict(gidx=gidx_all, ind=ind_all, pool=pool_all, mask=mask_all,
                   x0T=x0T_all)
    return plan, percore


def _build(plan, flags, dbg=None):
    """Build the SPMD Bass program (identical across cores)."""
    cpw = plan["cpw"]
    KA, KB, K = plan["KA"], plan["KB"], plan["K"]
    if dbg is None:
        dbg = {}
    NL = dbg.get("nl", L)          # layers to actually run
    DO_FFN = dbg.get("ffn", True)  # run FFN block per group
    DO_TAIL = dbg.get("tail", True)  # stats AR + BN apply + pool + AG
    NO_MM = dbg.get("nomm", False)   # skip seg matmuls/evictions
    NO_IND = dbg.get("noind", False)  # skip indicator DMAs
    NO_CONST = dbg.get("noconst", False)  # skip ident/poolind/mask/eps/ones loads
    NO_PROJ = dbg.get("noproj", False)  # skip proj matmuls (raw x -> table)
    NO_LONGPS = dbg.get("nolongps", False)  # skip stats/pool psum allocation
    DVE_MM = dbg.get("dvemm", False)  # replace seg MMs with DVE consumes
    DUMP = dbg.get("dump", False)    # dump stage intermediates (layer 0)

    nc = bacc.Bacc("TRN2", target_bir_lowering=False, num_swdge_queues=4)

    # ---- DRAM parameters ----
    p_x0T = nc.declare_dram_parameter("x0T", [128, NPAD], BF16, isOutput=False)
    p_gidx = nc.declare_dram_parameter("gidx", [128, K * 8], I16, isOutput=False)
    p_ind = nc.declare_dram_parameter("ind", [128, K, W], BF16, isOutput=False)
    p_pool = nc.declare_dram_parameter("poolind", [128, NGRP, 2, 128], BF16,
                                       isOutput=False)
    p_mask = nc.declare_dram_parameter("mask", [128, NGRP], BF16, isOutput=False)
    p_wlin = nc.declare_dram_parameter("wlin", [128, 128], BF16, isOutput=False)
    p_blin = nc.declare_dram_parameter("blin", [1, 128], BF16, isOutput=False)
    p_W1 = nc.declare_dram_parameter("W1", [L, 128, 256], BF16, isOutput=False)
    p_b1 = nc.declare_dram_parameter("b1", [L, 1, 256], BF16, isOutput=False)
    # W2 stored as [L, 128, 2, 128]: [:, p, k, :] = W2[l, k*128+p, :]
    p_W2 = nc.declare_dram_parameter("W2", [L, 128, 2, 128], BF16, isOutput=False)
    p_b2 = nc.declare_dram_parameter("b2", [L, 1, 128], BF16, isOutput=False)
    p_lng = nc.declare_dram_parameter("lng", [L, 128, 256], F32, isOutput=False)
    p_lnb = nc.declare_dram_parameter("lnb", [L, 128, 256], F32, isOutput=False)
    p_bng = nc.declare_dram_parameter("bng", [L, 1, 128], F32, isOutput=False)
    p_bnb = nc.declare_dram_parameter("bnb", [L, 1, 128], F32, isOutput=False)
    o_xs = nc.declare_dram_parameter("xs_out", [L, 128, NGRP, 128], F32,
                                     isOutput=True)
    o_pool = nc.declare_dram_parameter("pool_out", [L, 2, 128, 128], F32,
                                       isOutput=True)
    if dbg and dbg.get("dump", False):
        o_dtbl = nc.declare_dram_parameter("dbg_tbl", [TBL, 128], BF16,
                                           isOutput=True)
        o_daggr = nc.declare_dram_parameter("dbg_aggrA", [128, NWIN * W], F32,
                                            isOutput=True)
        o_dysb = nc.declare_dram_parameter("dbg_ysb", [128, NGRP, 128], F32,
                                           isOutput=True)

    # internal DRAM
    ag_in = nc.dram_tensor("ag_in", [NPAD, 128], BF16)
    tables = [nc.dram_tensor(f"table{l}", [TBL, 128], BF16, addr_space="Shared")
              for l in range(L)]
    ar_ins = [nc.dram_tensor(f"ar_in{l}", [1, 256], F32) for l in range(L)]
    ar_outs = [nc.dram_tensor(f"ar_out{l}", [1, 256], F32, addr_space="Shared")
               for l in range(L)]

    RG = [list(range(C))]

    # chunk metadata: list over k of (bucket, window)
    chunk_win = np.zeros(K, np.int64)
    for b in range(2):
        for w in range(NWIN):
            base = plan["chunk_base"][b, w]
            chunk_win[base:base + cpw[b, w]] = w
    # supercalls within each bucket
    def supercalls(lo, hi):
        out = []
        k = lo
        while k < hi:
            out.append((k, min(SC, hi - k)))
            k += SC
        return out

    calls_A = supercalls(0, KA)
    calls_B = supercalls(KA, K)

    with TileContext(nc) as tc:
        with (
            tc.tile_pool(name="const", bufs=1) as cpool,
            tc.tile_pool(name="params", bufs=2) as ppool,
            tc.tile_pool(name="indp", bufs=2) as indp,
            tc.tile_pool(name="goutp", bufs=2) as goutp,
            tc.tile_pool(name="segps", bufs=2, space="PSUM") as segps,
            tc.tile_pool(name="ffnps", bufs=2, space="PSUM") as ffnps,
            tc.tile_pool(name="longps", bufs=1, space="PSUM") as longps,
            tc.tile_pool(name="work", bufs=3) as work,
            tc.tile_pool(name="bigbuf", bufs=1) as bigp,
        ):
            # ---- constants ----
            ident = cpool.tile([128, 128], BF16)
            ones_bf = cpool.tile([1, 128], BF16)
            ones_f = cpool.tile([1, 128], F32)
            epsc = cpool.tile([128, 1], F32)
            poolind = cpool.tile([128, NGRP, 2, 128], BF16)
            maskt = cpool.tile([128, NGRP], BF16)
            if not NO_CONST:
                make_identity(nc, ident[:, :])
                nc.vector.memset(ones_bf[:, :], 1.0)
                nc.vector.memset(ones_f[:, :], 1.0)
                nc.vector.memset(epsc[:, :], EPS)
                nc.sync.dma_start(out=poolind[:, :, :, :], in_=p_pool[:, :, :, :])
                nc.sync.dma_start(out=maskt[:, :], in_=p_mask[:, :])
            gidx = cpool.tile([128, K * 8], I16)
            nc.sync.dma_start(out=gidx[:, :], in_=p_gidx[:, :])
            wlin = cpool.tile([128, 128], BF16)
            nc.sync.dma_start(out=wlin[:, :], in_=p_wlin[:, :])
            blin = cpool.tile([1, 128], BF16)
            nc.sync.dma_start(out=blin[:, :], in_=p_blin[:, :])

            # ---- projection: x1 = x0 @ lin_W + lin_b -> bf16 table0 ----
            x0T = bigp.tile([128, NPAD], BF16, tag="x0T")
            nc.sync.dma_start(out=x0T[:, :], in_=p_x0T[:, :])
            xnew_bf = bigp.tile([128, NGRP, 128], BF16, tag="xnewbf")
            for t in range(0 if NO_PROJ else NGRP):
                ps = ffnps.tile([128, 128], F32, tag="hps")
                nc.tensor.matmul(ps[:, :], lhsT=x0T[:, t * 128:(t + 1) * 128],
                                 rhs=wlin[:, :], start=True,
                                 stop=flags["blin_zero"])
                if not flags["blin_zero"]:
                    nc.tensor.matmul(ps[:, :], lhsT=ones_bf[:, :],
                                     rhs=blin[:, :], start=False, stop=True)
                nc.scalar.copy(out=xnew_bf[:, t, :], in_=ps[:, :])
            if NO_PROJ:
                nc.sync.dma_start(
                    out=ag_in[:, :].rearrange("(t p) f -> p (t f)", p=128),
                    in_=x0T[:, :],
                )
            else:
                nc.sync.dma_start(
                    out=ag_in[:, :].rearrange("(t p) f -> p t f", p=128),
                    in_=xnew_bf[:, :, :],
                )
            ag0 = nc.gpsimd.collective_compute(
                "AllGather", ALU.bypass, replica_groups=RG,
                ins=[ag_in.ap().opt()], outs=[tables[0].ap().opt()],
            )
            ag_insts = [ag0, None, None]
            if DUMP:
                tcp = nc.sync.dma_start(out=o_dtbl[:, :], in_=tables[0][:, :])
                add_dep_helper(tcp.ins, ag0.ins, sync=True, reason="dbg tbl dump")

            # ---- layers ----
            aggrA = bigp.tile([128, NWIN * W], F32, tag="aggrA")
            y_sb = bigp.tile([128, NGRP, 128], F32, tag="ysb")
            xnew_f = bigp.tile([128, NGRP, 128], F32, tag="xnewf")

            for l in range(NL):
                table = tables[l]
                tblA = table[0:BSPLIT, :]
                tblB = table[BSPLIT:TBL, :]

                w1 = ppool.tile([128, 256], BF16, tag="w1")
                nc.sync.dma_start(out=w1[:, :], in_=p_W1[l, :, :])
                w2 = ppool.tile([128, 2, 128], BF16, tag="w2")
                nc.sync.dma_start(out=w2[:, :, :], in_=p_W2[l, :, :, :])
                b1r = ppool.tile([1, 256], BF16, tag="b1r")
                nc.sync.dma_start(out=b1r[:, :], in_=p_b1[l, :, :])
                b2r = ppool.tile([1, 128], BF16, tag="b2r")
                nc.sync.dma_start(out=b2r[:, :], in_=p_b2[l, :, :])
                if not flags["ln_triv"]:
                    lng = ppool.tile([128, 256], F32, tag="lng")
                    nc.sync.dma_start(out=lng[:, :], in_=p_lng[l, :, :])
                    lnb = ppool.tile([128, 256], F32, tag="lnb")
                    nc.sync.dma_start(out=lnb[:, :], in_=p_lnb[l, :, :])
                bng = ppool.tile([1, 128], F32, tag="bng")
                nc.sync.dma_start(out=bng[:, :], in_=p_bng[l, :, :])
                bnb = ppool.tile([1, 128], F32, tag="bnb")
                nc.sync.dma_start(out=bnb[:, :], in_=p_bnb[l, :, :])

                if not NO_LONGPS:
                    stats_ps = longps.tile([1, 256], F32, tag="stats")
                    pool_ps = longps.tile([128, 256], F32, tag="poolps")

                # ---------- seg-matmul passes ----------
                def run_pass(calls, tbl_ap, pass_b, dep_inst):
                    cur_ps = {}

                    def win_ps(w):
                        g = w // WPG
                        if g not in cur_ps:
                            cur_ps[g] = segps.tile([128, 128], F32, tag="seg", name=f"seg{g}")
                        return cur_ps[g], (w % WPG) * W

                    done_in_win = {}
                    for (k0, nch) in calls:
                        gout = goutp.tile([128, SC, 128], BF16, tag="gout")
                        indt = indp.tile([128, SC, W], BF16, tag="ind")
                        if not NO_IND:
                            nc.sync.dma_start(out=indt[:, 0:nch, :],
                                              in_=p_ind[:, k0:k0 + nch, :])
                        g_inst = nc.gpsimd.dma_gather(
                            out_ap=gout[:, 0:nch, :],
                            in_ap=tbl_ap,
                            idxs_ap=gidx[:, k0 * 8:(k0 + nch) * 8],
                            num_idxs=nch * 128,
                            num_idxs_reg=nch * 128,
                            elem_size=128,
                            queue_num=(k0 // SC) % 4,
                            single_packet=False,
                        )
                        if dep_inst is not None:
                            add_dep_helper(g_inst.ins, dep_inst.ins, sync=True,
                                           reason="gather reads AG table")
                        if NO_MM:
                            continue
                        for j in range(nch):
                            k = k0 + j
                            w = int(chunk_win[k])
                            b = 1 if pass_b else 0
                            first = done_in_win.get(w, 0) == 0
                            done_in_win[w] = done_in_win.get(w, 0) + 1
                            last = done_in_win[w] == cpw[b, w]
                            if DVE_MM:
                                sc1 = work.tile([128, 64], F32, tag="dvemm",
                                                name=f"dv{k}{pass_b}")
                                nc.vector.tensor_tensor(
                                    out=sc1[:, :], in0=gout[:, j, 0:64],
                                    in1=indt[:, j, :], op=ALU.add)
                                if last and w % WPG == 1:
                                    yield w // WPG, None
                                continue
                            ps, coff = win_ps(w)
                            # PSUM start=True zeroes the whole 2KB bank, so
                            # only the first matmul touching the tile starts.
                            nc.tensor.matmul(
                                ps[:, coff:coff + W],
                                lhsT=gout[:, j, :],
                                rhs=indt[:, j, :],
                                start=(first and w % WPG == 0),
                                stop=(last and w % WPG == 1),
                                skip_group_check=True,
                            )
                            if last and w % WPG == 1:
                                g = w // WPG
                                yield g, cur_ps.pop(g)

                # pass A: evict into aggrA
                for g, ps in run_pass(calls_A, tblA, False, ag_insts[l]):
                    if ps is None:
                        continue
                    nc.scalar.copy(out=aggrA[:, g * 128:(g + 1) * 128],
                                   in_=ps[:, :])

                if DUMP and l == 0:
                    nc.sync.dma_start(out=o_daggr[:, :], in_=aggrA[:, :])
                # pass B: combine + FFN per group
                for g, psB in run_pass(calls_B, tblB, True, ag_insts[l]):
                    aggrT_f = work.tile([128, 128], F32, tag="aggrTf")
                    if psB is None:
                        nc.vector.memset(aggrT_f[:, :], 0.0)
                    else:
                        nc.vector.tensor_tensor(
                            out=aggrT_f[:, :], in0=psB[:, :],
                            in1=aggrA[:, g * 128:(g + 1) * 128], op=ALU.add)
                    if not DO_FFN:
                        nc.scalar.copy(out=y_sb[:, g, :], in_=aggrT_f[:, :])
                        continue
                    aggrT_bf = work.tile([128, 128], BF16, tag="aggrTbf")
                    nc.scalar.copy(out=aggrT_bf[:, :], in_=aggrT_f[:, :])

                    h_ps = ffnps.tile([128, 256], F32, tag="hps")
                    nc.tensor.matmul(h_ps[:, :], lhsT=aggrT_bf[:, :],
                                     rhs=w1[:, :], start=True,
                                     stop=flags["b1_zero"])
                    if not flags["b1_zero"]:
                        nc.tensor.matmul(h_ps[:, :], lhsT=ones_bf[:, :],
                                         rhs=b1r[:, :], start=False, stop=True)
                    st6 = work.tile([128, 6], F32, tag="st6")
                    nc.vector.bn_stats(st6[:, :], h_ps[:, :])
                    mv = work.tile([128, 2], F32, tag="mv")
                    nc.vector.bn_aggr(mv[:, :], st6[:, :])
                    sd = work.tile([128, 1], F32, tag="sd")
                    nc.scalar.activation(sd[:, :], mv[:, 1:2], AF.Sqrt, bias=epsc[:, :])
                    inv = work.tile([128, 1], F32, tag="inv")
                    nc.vector.reciprocal(inv[:, :], sd[:, :])
                    t_sb = work.tile([128, 256], F32, tag="tsb")
                    nc.vector.tensor_scalar(
                        out=t_sb[:, :], in0=h_ps[:, :],
                        scalar1=mv[:, 0:1], scalar2=inv[:, :],
                        op0=ALU.subtract, op1=ALU.mult)
                    if not flags["ln_triv"]:
                        u = work.tile([128, 256], F32, tag="lnu")
                        nc.vector.tensor_tensor(out=u[:, :], in0=t_sb[:, :],
                                                in1=lng[:, :], op=ALU.mult)
                        nc.vector.tensor_tensor(out=u[:, :], in0=u[:, :],
                                                in1=lnb[:, :], op=ALU.add)
                        relu_src = u
                    else:
                        relu_src = t_sb
                    rh_bf = work.tile([128, 256], BF16, tag="rhbf")
                    nc.scalar.activation(rh_bf[:, :], relu_src[:, :], AF.Relu)

                    tr0 = ffnps.tile([128, 128], BF16, tag="fsm")
                    nc.tensor.transpose(tr0[:, :], rh_bf[:, 0:128], ident[:, :])
                    tr1 = ffnps.tile([128, 128], BF16, tag="fsm")
                    nc.tensor.transpose(tr1[:, :], rh_bf[:, 128:256], ident[:, :])
                    rhT0 = work.tile([128, 128], BF16, tag="rhT0")
                    nc.vector.tensor_copy(out=rhT0[:, :], in_=tr0[:, :])
                    rhT1 = work.tile([128, 128], BF16, tag="rhT1")
                    nc.scalar.copy(out=rhT1[:, :], in_=tr1[:, :])

                    y_ps = ffnps.tile([128, 128], F32, tag="fsm")
                    nc.tensor.matmul(y_ps[:, :], lhsT=rhT0[:, :],
                                     rhs=w2[:, 0, :], start=True, stop=False)
                    nc.tensor.matmul(y_ps[:, :], lhsT=rhT1[:, :],
                                     rhs=w2[:, 1, :], start=False, stop=False)
                    if not flags["b2_zero"]:
                        nc.tensor.matmul(y_ps[:, :], lhsT=ones_bf[:, :],
                                         rhs=b2r[:, :], start=False, stop=False)
                    nc.tensor.matmul(y_ps[:, :], lhsT=aggrT_bf[:, :],
                                     rhs=ident[:, :], start=False, stop=True)

                    nc.scalar.activation(y_sb[:, g, :], y_ps[:, :], AF.Relu)
                    ybf = work.tile([128, 128], BF16, tag="ybf")
                    nc.scalar.copy(out=ybf[:, :], in_=y_sb[:, g, :])
                    ysq = work.tile([128, 128], BF16, tag="ysq")
                    nc.scalar.activation(ysq[:, :], y_sb[:, g, :], AF.Square)
                    nc.tensor.matmul(stats_ps[:, 0:128],
                                     lhsT=maskt[:, g:g + 1], rhs=ybf[:, :],
                                     start=(g == 0), stop=False,
                                     skip_group_check=True)
                    nc.tensor.matmul(stats_ps[:, 128:256],
                                     lhsT=maskt[:, g:g + 1], rhs=ysq[:, :],
                                     start=False, stop=(g == NGRP - 1),
                                     skip_group_check=True)

                if DUMP and l == 0:
                    nc.sync.dma_start(out=o_dysb[:, :, :], in_=y_sb[:, :, :])
                # ---------- BN stats all-reduce ----------
                if not DO_TAIL:
                    nc.sync.dma_start(out=o_xs[l, :, :, :], in_=y_sb[:, :, :])
                    continue
                st_sb = work.tile([1, 256], F32, tag="stsb")
                nc.scalar.copy(out=st_sb[:, :], in_=stats_ps[:, :])
                nc.gpsimd.dma_start(out=ar_ins[l][:, :], in_=st_sb[:, :])
                ar_inst = nc.gpsimd.collective_compute(
                    "AllReduce", ALU.add, replica_groups=RG,
                    ins=[ar_ins[l].ap().opt()], outs=[ar_outs[l].ap().opt()],
                )
                stg = work.tile([1, 256], F32, tag="stg")
                stg_rd = nc.sync.dma_start(out=stg[:, :], in_=ar_outs[l][:, :])
                add_dep_helper(stg_rd.ins, ar_inst.ins, sync=True,
                               reason="stats read after AllReduce")
                st2 = work.tile([1, 256], F32, tag="st2")  # [s | t]
                mrow = work.tile([1, 128], F32, tag="mrow")
                nc.vector.tensor_scalar_mul(mrow[:, :], stg[:, 0:128], 1.0 / N)
                qrow = work.tile([1, 128], F32, tag="qrow")
                nc.vector.tensor_scalar_mul(qrow[:, :], stg[:, 128:256], 1.0 / N)
                msq = work.tile([1, 128], F32, tag="msq")
                nc.vector.tensor_tensor(out=msq[:, :], in0=mrow[:, :],
                                        in1=mrow[:, :], op=ALU.mult)
                vrow = work.tile([1, 128], F32, tag="vrow")
                nc.vector.tensor_tensor(out=vrow[:, :], in0=qrow[:, :],
                                        in1=msq[:, :], op=ALU.subtract)
                sdr = work.tile([1, 128], F32, tag="sdr")
                nc.scalar.activation(sdr[:, :], vrow[:, :], AF.Sqrt, bias=epsc[0:1, :])
                invr = work.tile([1, 128], F32, tag="invr")
                nc.vector.reciprocal(invr[:, :], sdr[:, :])
                if flags["bng_one"]:
                    nc.vector.tensor_copy(out=st2[:, 0:128], in_=invr[:, :])
                else:
                    nc.vector.tensor_tensor(out=st2[:, 0:128], in0=invr[:, :],
                                            in1=bng[:, :], op=ALU.mult)
                ms = work.tile([1, 128], F32, tag="ms")
                nc.vector.tensor_tensor(out=ms[:, :], in0=mrow[:, :],
                                        in1=st2[:, 0:128], op=ALU.mult)
                if flags["bnb_zero"]:
                    nc.vector.tensor_scalar_mul(st2[:, 128:256], ms[:, :], -1.0)
                else:
                    nc.vector.tensor_tensor(out=st2[:, 128:256], in0=bnb[:, :],
                                            in1=ms[:, :], op=ALU.subtract)
                bc_ps = ffnps.tile([128, 256], F32, tag="hps")
                nc.tensor.matmul(bc_ps[:, :], lhsT=ones_f[:, :], rhs=st2[:, :],
                                 start=True, stop=True)
                st_bc = work.tile([128, 256], F32, tag="stbc")
                nc.scalar.copy(out=st_bc[:, :], in_=bc_ps[:, :])

                # ---------- BN apply + pool ----------
                for g in range(NGRP):
                    tmp = work.tile([128, 128], F32, tag="bntmp")
                    nc.vector.tensor_tensor(out=tmp[:, :], in0=y_sb[:, g, :],
                                            in1=st_bc[:, 0:128], op=ALU.mult)
                    nc.vector.tensor_tensor(out=xnew_f[:, g, :], in0=tmp[:, :],
                                            in1=st_bc[:, 128:256], op=ALU.add)
                    nc.scalar.copy(out=xnew_bf[:, g, :], in_=xnew_f[:, g, :])
                    nc.tensor.matmul(pool_ps[:, 0:128], lhsT=poolind[:, g, 0, :],
                                     rhs=xnew_bf[:, g, :], start=(g == 0),
                                     stop=False, skip_group_check=True)
                    nc.tensor.matmul(pool_ps[:, 128:256], lhsT=poolind[:, g, 1, :],
                                     rhs=xnew_bf[:, g, :], start=False,
                                     stop=(g == NGRP - 1), skip_group_check=True)

                nc.sync.dma_start(out=o_xs[l, :, :, :], in_=xnew_f[:, :, :])
                pl0 = work.tile([128, 128], F32, tag="pl0")
                nc.scalar.copy(out=pl0[:, :], in_=pool_ps[:, 0:128])
                nc.sync.dma_start(out=o_pool[l, 0, :, :], in_=pl0[:, :])
                pl1 = work.tile([128, 128], F32, tag="pl1")
                nc.scalar.copy(out=pl1[:, :], in_=pool_ps[:, 128:256])
                nc.sync.dma_start(out=o_pool[l, 1, :, :], in_=pl1[:, :])

                if l < L - 1:
                    nc.sync.dma_start(
                        out=ag_in[:, :].rearrange("(t p) f -> p t f", p=128),
                        in_=xnew_bf[:, :, :],
                    )
                    ag_insts[l + 1] = nc.gpsimd.collective_compute(
                        "AllGather", ALU.bypass, replica_groups=RG,
                        ins=[ag_in.ap().opt()],
                        outs=[tables[l + 1].ap().opt()],
                    )

    nc.compile()
    return nc


def kernel(**inputs):
    x = _to_np(inputs["x"], np.float32)
    ei = _to_np(inputs["edge_index"], np.int64)
    batch = _to_np(inputs["batch"], np.int64)
    ew = _to_np(inputs["edge_weight"], np.float32)
    lin_W = _to_np(inputs["lin_W"], np.float32)
    lin_b = _to_np(inputs["lin_b"], np.float32)
    W1 = _to_np(inputs["W1"], np.float32)
    b1 = _to_np(inputs["b1"], np.float32)
    ln_gamma = _to_np(inputs["ln_gamma"], np.float32)
    ln_beta = _to_np(inputs["ln_beta"], np.float32)
    W2 = _to_np(inputs["W2"], np.float32)
    b2 = _to_np(inputs["b2"], np.float32)
    bn_gamma = _to_np(inputs["bn_gamma"], np.float32)
    bn_beta = _to_np(inputs["bn_beta"], np.float32)

    src, dst = ei[0], ei[1]
    plan, percore = _prep(x, src, dst, ew, batch)

    flags = dict(
        blin_zero=bool(np.all(lin_b == 0)),
        b1_zero=bool(np.all(b1 == 0)),
        b2_zero=bool(np.all(b2 == 0)),
        ln_triv=bool(np.all(ln_gamma == 1) and np.all(ln_beta == 0)),
        bng_one=bool(np.all(bn_gamma == 1)),
        bnb_zero=bool(np.all(bn_beta == 0)),
    )

    nc = _build(plan, flags)

    shared = dict(
        wlin=lin_W.astype(NP_BF16),
        blin=lin_b.reshape(1, 128).astype(NP_BF16),
        W1=W1.astype(NP_BF16),
        b1=b1.reshape(L, 1, 256).astype(NP_BF16),
        W2=W2.reshape(L, 2, 128, 128).transpose(0, 2, 1, 3).copy().astype(NP_BF16),
        b2=b2.reshape(L, 1, 128).astype(NP_BF16),
        lng=np.broadcast_to(ln_gamma.reshape(L, 1, 256),
                            (L, 128, 256)).copy().astype(np.float32),
        lnb=np.broadcast_to(ln_beta.reshape(L, 1, 256),
                            (L, 128, 256)).copy().astype(np.float32),
        bng=bn_gamma.reshape(L, 1, 128).astype(np.float32),
        bnb=bn_beta.reshape(L, 1, 128).astype(np.float32),
    )
    in_maps = []
    for c in range(C):
        m = dict(shared)
        m["x0T"] = percore["x0T"][c]
        m["gidx"] = percore["gidx"][c]
        m["ind"] = percore["ind"][c]
        m["poolind"] = percore["pool"][c]
        m["mask"] = percore["mask"][c]
        in_maps.append(m)

    res = run_bass_kernel_spmd(nc, in_maps, core_ids=list(range(C)))
    global _last_run
    _last_run = res

    xs = np.zeros((N, L * DIM), np.float32)
    pooled = np.zeros((G, L * DIM), np.float32)
    for c in range(C):
        r = res.results[c]
        xo = r["xs_out"]       # [L, 128, NGRP, 128]
        po = r["pool_out"]     # [L, 2, 128, 128]
        for l in range(L):
            sl = xo[l].transpose(1, 0, 2).reshape(NPAD, 128)[:NPC]
            xs[c * NPC:(c + 1) * NPC, l * DIM:(l + 1) * DIM] = sl
            pooled[:, l * DIM:(l + 1) * DIM] += po[l].reshape(256, 128)
    return pooled, xs
